# revision 47
# baseline (speedup 1.0000x reference)
"""AttVlad Trainium2 kernel.

Math (per image n):
  xn = x / ||x||_2(over d)                       x: [D=128, S]
  a  = softmax_k(conv_w @ xn + conv_b)           a: [K=64, S]
  vlad[k,d]   = sum_s a[k,s] xn[d,s] - (sum_s a[k,s]) centroids[k,d]
  out = normalize_d(vlad * (centroids @ att_w.T + att_b))

Device strategy (8 cores, data-parallel over n, 4 images each):
  - x is cast to bf16 on the host (the device math is bf16 either way, so
    this is bit-identical) and streamed in [128d, 4096s] HWDGE chunks,
    halving device HBM traffic.
  - Per 128-s unit: one PE pass with lhsT = x_chunk slice produces BOTH
    logits^T [128s, 64k] (rhs = conv_w^T) and x^T [128s, 128d] (rhs = I).
  - All per-s scalars (rsqrt of sumsq, softmax denom, their products) live
    as [128, 16] tiles (s on partitions) and are applied via broadcast
    (step-0) access patterns, so softmax needs no per-unit scalar ops.
  - Normalization scalars never touch x: logits are scaled by rnorm before
    exp; the VLAD matmul uses lhsT a2 = exp(l*rnorm)*exp(b) * (rnorm*rdenom)
    and rhs = [x^T | norm], giving columns [A | asum] accumulated in PSUM.
  - rsqrt is computed as exp(-0.5*ln(s)) to stay inside one ACT table set.
  - Host does the O(N*K*D) finalize (centroid subtract, attention scale,
    intra-normalization) in float64.
"""

import sys
import time

import numpy as np

try:  # the concourse stack (bass) ships in the container image
    import concourse.bass as _probe  # noqa: F401
except Exception:  # pragma: no cover
    sys.path.insert(0, "/opt/trn_rl_repo")

import ml_dtypes

N, D, S, K = 32, 128, 16384, 64
NCORES = 8
EPS = 1e-12

CHUNK = 4096  # s-positions per DMA chunk
UNIT = 128    # s-positions per matmul unit (psum partition dim)
XT_STRIDE = 130  # x^T unit stride in the SBUF tile: 128 cols x^T + 1 norm + 1 pad


def _make_tile_context_cls(tile, mybir, ScopedClock):
    """This walrus build rejects instructions carrying more than one sync
    wait; excess waits are split onto same-engine NoOps by _split_waits."""
    return tile.TileContext


# this walrus build rejects >1 sync wait on every instruction struct probed
# (CTRL, TT, MM); keep both caps at 1
MAX_WAITS = 1
COMPUTE_WAITS = 1
_COMPUTE_TYPES = (
    "InstTensorTensor", "InstActivation", "InstMatmult", "InstTensorReduce",
    "InstReciprocal", "InstTensorCopy", "InstLdweights", "InstTensorScalarPtr",
    "InstMemSet", "InstTensorScalar",
)


def _split_waits(nc, mybir):
    """Rewrite the traced BIR so no instruction carries more sem waits than
    this walrus build's per-struct limit: excess waits move to injected NoOps
    immediately preceding the instruction on the same engine (NX executes
    waits in order, so this is semantically identical)."""
    nid = 0
    for f in nc.m.functions:
        for blk in f.blocks:
            new_insts = []
            for inst in blk.instructions:
                si = getattr(inst, "sync_info", None)
                ws = list(si.on_wait) if si is not None else []
                maxw = (
                    COMPUTE_WAITS
                    if type(inst).__name__ in _COMPUTE_TYPES
                    else MAX_WAITS
                )
                if len(ws) > maxw:
                    extra = ws[: len(ws) - maxw]
                    for i in range(0, len(extra), MAX_WAITS):
                        nid += 1
                        nop = mybir.InstNoOp(
                            name=f"waitsplit_{nid}", ins=[], outs=[]
                        )
                        nop.engine = inst.engine
                        nop.sync_info = mybir.SyncInfo(
                            on_wait=extra[i : i + MAX_WAITS], on_update=[]
                        )
                        new_insts.append(nop)
                    si.on_wait = ws[len(ws) - maxw :]
                new_insts.append(inst)
            blk.instructions[:] = new_insts


# tunables (engine assignment variants, sweepable via the cost model)
OPT_SUMSQ = "pe"      # "dve": square+reduce on DVE | "pe": natural-layout square + ones-matmul
OPT_SOFT = "split"   # engine for a_un/a2 elementwise muls: "dve" | "gpsimd"


def build_program(n_per_core=4, s_total=S, reps=1, n_read=None):
    """Build the single-core Bass program (same program runs on all cores).
    reps>1 repeats the whole computation; n_read<n_per_core processes only
    the first n_read images (input shapes unchanged) — both are for
    slope-based HW timing."""
    if n_read is None:
        n_read = n_per_core
    import concourse.bass as bass
    import concourse.tile as tile
    from concourse import mybir
    from concourse.vector_clock import ScopedClock

    dt = mybir.dt
    AF = mybir.ActivationFunctionType
    OP = mybir.AluOpType

    TileContextFixed = _make_tile_context_cls(tile, mybir, ScopedClock)

    n_chunks = s_total // CHUNK
    units_per_chunk = CHUNK // UNIT
    HU = 8                                   # units per group (psum-bank sized)
    halves = tuple(range(units_per_chunk // HU))

    nc = bass.Bass()
    x_in = nc.declare_dram_parameter(
        "x", [n_per_core, D, s_total], dt.bfloat16, isOutput=False
    )
    wt_in = nc.declare_dram_parameter("wt", [D, K], dt.bfloat16, isOutput=False)
    idm_in = nc.declare_dram_parameter("idm", [D, D], dt.bfloat16, isOutput=False)
    bseed_in = nc.declare_dram_parameter(
        "bseed", [1, 2 * K], dt.float8e4, isOutput=False
    )
    ones_in = nc.declare_dram_parameter("ones", [D, 1], dt.bfloat16, isOutput=False)
    out_dram = nc.declare_dram_parameter(
        "out", [K, n_per_core * 132], dt.float32, isOutput=True
    )

    with TileContextFixed(nc) as tc:
        with (
            tc.tile_pool(name="consts", bufs=1) as consts,
            tc.tile_pool(name="xc", bufs=4) as xc_pool,
            tc.tile_pool(name="xt", bufs=4) as xt_pool,
            tc.tile_pool(name="soft", bufs=7) as soft_pool,
            tc.tile_pool(name="stats", bufs=4) as stats_pool,
            tc.tile_pool(name="scratch", bufs=3) as scratch_pool,
            tc.tile_pool(name="seed", bufs=3) as seed_pool,
            tc.tile_pool(name="outp", bufs=1) as out_pool,
            tc.tile_pool(name="psl", bufs=3, space="PSUM") as psl_pool,
            tc.tile_pool(name="pst", bufs=1, space="PSUM") as pst_pool,
            tc.tile_pool(name="pv", bufs=2, space="PSUM") as pv_pool,
            tc.tile_pool(name="pss", bufs=1, space="PSUM") as pss_pool,
        ):
            wt = consts.tile([D, K], dt.bfloat16)
            nc.sync.dma_start(wt[:], wt_in[:])
            idm = consts.tile([D, D], dt.bfloat16)
            nc.sync.dma_start(idm[:], idm_in[:])
            bseed = consts.tile([1, 2 * K], dt.float8e4)
            nc.sync.dma_start(bseed[:], bseed_in[:])
            bseed3 = bseed[:].rearrange("p (j n) -> p j n", j=2)
            ones = consts.tile([D, 1], dt.bfloat16)
            nc.sync.dma_start(ones[:], ones_in[:])

            out_sb = out_pool.tile([K, n_per_core * 132], dt.float32)
            # touch the ln/exp ACT table set immediately so its ~2.7us DMA
            # overlaps the initial input loads instead of the first chunk
            warm = consts.tile([1, 1], dt.float32)
            nc.scalar.activation(warm[:], ones[0:1, 0:1], AF.Ln)

            def emit_all():
              chunk_list = [
                  (n, ci) for n in range(n_read) for ci in range(n_chunks)
              ]
              lead_state = {}
              pv_state = {}

              def lead(n, ci):
                  """Per-chunk stats lead-in: load, square, per-unit sumsq
                  matmuls, rnorm. Emitted one chunk ahead of main() so the
                  baked in-order engine schedules interleave the next chunk's
                  lead-in with this chunk's softmax (no head-of-line block)."""
                  xc = xc_pool.tile([D, CHUNK], dt.bfloat16, name="xc")
                  # x is pre-cast to bf16 on the host (identical math to an
                  # on-device cast, half the HBM traffic). The very first
                  # chunk loads in quarters so compute starts ~2us sooner.
                  first_chunk = (n, ci) == (0, 0)
                  QC = CHUNK // 4
                  if first_chunk:
                      for q in range(4):
                          nc.gpsimd.dma_start(
                              xc[:, q * QC : (q + 1) * QC],
                              x_in[n, :, ci * CHUNK + q * QC
                                   : ci * CHUNK + (q + 1) * QC],
                          )
                  else:
                      nc.gpsimd.dma_start(
                          xc[:], x_in[n, :, ci * CHUNK : (ci + 1) * CHUNK]
                      )
                  rn = stats_pool.tile(
                      [128, units_per_chunk], dt.float32, tag="rn", name="rn"
                  )
                  lns = stats_pool.tile(
                      [128, units_per_chunk], dt.float32, tag="lns", name="lns"
                  )
                  # sumsq via PE: square x in natural layout, then per unit
                  # contract over d with a ones column, landing sumsq directly
                  # in s-partition orientation in PSUM.
                  xsq = scratch_pool.tile(
                      [D, CHUNK], dt.bfloat16, tag="xsq", name="xsq"
                  )
                  if first_chunk:
                      for q in range(4):
                          nc.vector.tensor_tensor(
                              out=xsq[:, q * QC : (q + 1) * QC],
                              in0=xc[:, q * QC : (q + 1) * QC],
                              in1=xc[:, q * QC : (q + 1) * QC], op=OP.mult,
                          )
                  elif ci % 4 == 3:
                      # balance: a quarter of the squares run on ACT
                      nc.scalar.activation(xsq[:], xc[:], AF.Square)
                  else:
                      nc.vector.tensor_tensor(
                          out=xsq[:], in0=xc[:], in1=xc[:], op=OP.mult
                      )
                  pss = pss_pool.tile([128, 96], dt.float32, name="pss")
                  ss = pss[:, 0:32]
                  for cu in range(units_per_chunk):
                      nc.tensor.matmul(
                          ss[:, cu : cu + 1],
                          xsq[:, cu * UNIT : (cu + 1) * UNIT],
                          ones[:], start=True, stop=True,
                      )
                  # rnorm = exp(-0.5*ln(sumsq)); stays inside one ACT table set
                  nc.scalar.activation(lns[:], ss[:], AF.Ln)
                  nc.scalar.activation(rn[:], lns[:], AF.Exp, scale=-0.5)
                  # one x^T tile per chunk; norm column (asum rhs) written now:
                  # norm = sqrt(sumsq) = exp(0.5*ln)
                  xt = xt_pool.tile(
                      [128, units_per_chunk * XT_STRIDE], dt.bfloat16, name="xt"
                  )
                  xt3 = xt[:].rearrange("p (u c) -> p u c", c=XT_STRIDE)
                  # fp8 seed row: norm = sqrt(ss) compact, PE-transposed to
                  # [32u, 128s], then partition-collapsed to one row so the
                  # per-unit rank-1 bias matmuls (norm[s]*b[k], DoubleRow)
                  # can use it as a base-partition-0 stationary
                  nr16 = stats_pool.tile(
                      [128, units_per_chunk], dt.bfloat16, tag="nr16",
                      name="nr16"
                  )
                  nc.scalar.activation(nr16[:], lns[:], AF.Exp, scale=0.5)
                  # asum column of xt copied from the compact norms on Pool
                  nc.gpsimd.tensor_copy(xt3[:, :, D : D + 1],
                                        nr16[:][:, :, None])
                  seedT = pss[0:32, 32:96].bitcast(dt.bfloat16)
                  nc.tensor.transpose(seedT, nr16[:], idm[:])
                  seed8 = seed_pool.tile(
                      [units_per_chunk, UNIT], dt.float8e4, tag="s8",
                      name="seed8"
                  )
                  nc.scalar.activation(seed8[:], seedT, AF.Copy)
                  seed8f = seed_pool.tile(
                      [1, units_per_chunk * UNIT], dt.float8e4, tag="s8f",
                      name="seed8f"
                  )
                  nc.sync.dma_start(seed8f[:], seed8[:])
                  lead_state[(n, ci)] = (xc, rn, xt, seed8f)

              def main(n, ci):
                  xc, rn, xt, seed8f = lead_state.pop((n, ci))
                  if ci == 0:
                      pv_state[n] = pv_pool.tile([K, 132], dt.float32, name="pv")
                  pv = pv_state[n]
                  xt3 = xt[:].rearrange("p (u c) -> p u c", c=XT_STRIDE)

                  psls = []
                  for h in halves:
                      psl = psl_pool.tile([128, HU * K], dt.float32, name="psl")
                      pst = pst_pool.tile([128, HU * D], dt.float32, name="pst")
                      for u in range(HU):
                          cu = h * HU + u
                          lhsT = xc[:, cu * UNIT : (cu + 1) * UNIT]
                          srow = seed8f[0:1, cu * UNIT : (cu + 1) * UNIT][
                              :, None, :
                          ]
                          nc.tensor.matmul(
                              psl[:, u * K : (u + 1) * K],
                              srow.broadcast_to([1, 2, UNIT]), bseed3,
                              start=True, stop=False,
                              perf_mode=mybir.MatmulPerfMode.DoubleRow,
                          )
                          nc.tensor.matmul(
                              psl[:, u * K : (u + 1) * K], lhsT, wt[:],
                              start=False, stop=True,
                          )
                          nc.tensor.matmul(
                              pst[:, u * D : (u + 1) * D], lhsT, idm[:],
                              start=True, stop=True,
                          )
                      # batched PSUM->SBUF move of x^T (bf16), strided per unit
                      xt3h = xt3[:, h * HU : (h + 1) * HU, :]
                      pst3 = pst[:].rearrange("p (u c) -> p u c", c=D)
                      nc.scalar.activation(xt3h[:, :, 0:D], pst3, AF.Copy)
                      psls.append(psl)

                  for h in halves:
                      psl = psls[h]
                      rnh = rn[:, h * HU : (h + 1) * HU]
                      # l_scaled = logits_raw * rnorm (broadcast over k)
                      lsc = soft_pool.tile(
                          [128, HU * K], dt.bfloat16, tag="lsc", name="lsc"
                      )
                      nc.vector.tensor_tensor(
                          out=lsc[:].rearrange("p (u k) -> p u k", k=K),
                          in0=psl[:].rearrange("p (u k) -> p u k", k=K),
                          in1=rnh.broadcast_to([128, HU, K]),
                          op=OP.mult,
                      )
                      e = soft_pool.tile(
                          [128, HU * K], dt.bfloat16, tag="e", name="e"
                      )
                      nc.scalar.activation(e[:], lsc[:], AF.Exp)
                      # bias already folded in via the PE seed: e includes
                      # exp(b), so the denominator reduces e directly
                      dn = stats_pool.tile([128, HU], dt.float32, tag="dn", name="dn")
                      nc.vector.tensor_reduce(
                          out=dn[:],
                          in_=e[:].rearrange("p (u k) -> p u k", k=K),
                          axis=mybir.AxisListType.X, op=OP.add,
                      )
                      rdn = stats_pool.tile(
                          [128, HU], dt.float32, tag="rdn", name="rdn"
                      )
                      nc.vector.reciprocal(rdn[:], dn[:])
                      cc = stats_pool.tile([128, HU], dt.float32, tag="cc", name="cc")
                      nc.gpsimd.tensor_tensor(
                          out=cc[:], in0=rnh, in1=rdn[:], op=OP.mult
                      )
                      # a2 = a_un * (rnorm * rdenom)
                      a2 = soft_pool.tile(
                          [128, HU * K], dt.bfloat16, tag="a2", name="a2"
                      )
                      if OPT_SOFT == "gpsimd_ccb" or (
                          OPT_SOFT == "split" and h % 2 == 1
                      ):
                          # materialize cc broadcast (gpsimd 1-input) so the
                          # a2 multiply runs in the DVE 2x bf16 mode
                          ccb = soft_pool.tile(
                              [128, HU * K], dt.bfloat16, tag="ccb", name="ccb"
                          )
                          nc.gpsimd.tensor_copy(
                              ccb[:].rearrange("p (u k) -> p u k", k=K),
                              cc[:].broadcast_to([128, HU, K]),
                          )
                          nc.vector.tensor_tensor(
                              out=a2[:], in0=e[:], in1=ccb[:], op=OP.mult
                          )
                      else:
                          nc.vector.tensor_tensor(
                              out=a2[:].rearrange("p (u k) -> p u k", k=K),
                              in0=e[:].rearrange("p (u k) -> p u k", k=K),
                              in1=cc[:].broadcast_to([128, HU, K]),
                              op=OP.mult,
                          )
                      # VLAD accumulation: pv[:, :129] += a2_u^T @ [x^T | norm]
                      for u in range(HU):
                          cu = ci * units_per_chunk + h * HU + u
                          first = cu == 0
                          last = cu == (s_total // UNIT) - 1
                          xoff = (h * HU + u) * XT_STRIDE
                          nc.tensor.matmul(
                              pv[:, 0 : D + 1],
                              a2[:, u * K : (u + 1) * K],
                              xt[:, xoff : xoff + D + 1],
                              start=first, stop=last,
                          )
                  if ci == n_chunks - 1:
                      # stash [A | asum] for this n and ship it immediately
                      # so only the last image's store sits in the tail
                      nc.scalar.activation(
                          out_sb[:, n * 132 : n * 132 + D + 1],
                          pv[:, 0 : D + 1], AF.Copy,
                      )
                      nc.sync.dma_start(
                          out_dram[:, n * 132 : n * 132 + D + 1],
                          out_sb[:, n * 132 : n * 132 + D + 1],
                      )

              lead(*chunk_list[0])
              for i, (n, ci) in enumerate(chunk_list):
                  if i + 1 < len(chunk_list):
                      lead(*chunk_list[i + 1])
                  main(n, ci)
            if reps > 1:
                with tc.For_i(0, reps, 1):
                    emit_all()
            else:
                emit_all()

    _split_waits(nc, mybir)
    return nc


_CACHE = {}


def _get_program(n_per_core, s_total, reps=1, n_read=None):
    key = (n_per_core, s_total, reps, n_read)
    if key not in _CACHE:
        _CACHE[key] = build_program(n_per_core, s_total, reps, n_read)
    return _CACHE[key]


def run_device(x, conv_w, conv_b, n_per_core=4, s_total=S, trace=False):
    """Run the device part. x: [NCORES*n_per_core, D, s_total] fp32.
    Returns (A [n, K, D], asum [n, K], bass_results)."""
    from concourse.bass_utils import run_bass_kernel_spmd

    nc = _get_program(n_per_core, s_total)

    bf16 = ml_dtypes.bfloat16
    f8 = ml_dtypes.float8_e4m3
    wt_np = np.ascontiguousarray(conv_w.T.astype(bf16))           # [D, K]
    idm_np = np.eye(D, dtype=bf16)                                 # [D, D]
    bseed_np = np.concatenate(
        [conv_b.astype(np.float32), np.zeros(K, np.float32)]
    ).reshape(1, 2 * K).astype(f8)

    ones_np = np.ones((D, 1), bf16)
    in_maps = []
    for c in range(NCORES):
        xc = np.ascontiguousarray(
            x[c * n_per_core : (c + 1) * n_per_core].astype(bf16)
        )
        in_maps.append(
            {"x": xc, "wt": wt_np, "idm": idm_np, "bseed": bseed_np,
             "ones": ones_np}
        )

    try:
        res = run_bass_kernel_spmd(
            nc, in_maps, list(range(NCORES)), trace=trace,
        )
    except Exception:
        # one retry: the device occasionally reports a transient
        # unrecoverable state right after a failed prior load
        time.sleep(2)
        res = run_bass_kernel_spmd(
            nc, in_maps, list(range(NCORES)), trace=trace,
        )

    n_total = NCORES * n_per_core
    A = np.empty((n_total, K, D), np.float64)
    asum = np.empty((n_total, K), np.float64)
    for c in range(NCORES):
        o = res.results[c]["out"]  # [K, n_per_core*132]
        for nl in range(n_per_core):
            blk = o[:, nl * 132 : nl * 132 + D + 1].astype(np.float64)
            A[c * n_per_core + nl] = blk[:, :D]
            asum[c * n_per_core + nl] = blk[:, D]
    return A, asum, res


def finalize(A, asum, centroids, att_w, att_b):
    cen = centroids.astype(np.float64)
    vlad = A - asum[:, :, None] * cen[None]
    soft = cen @ att_w.astype(np.float64).T + att_b.astype(np.float64)  # [K, 1]
    av = vlad * soft[None]
    nrm = np.maximum(np.linalg.norm(av, axis=2, keepdims=True), EPS)
    return (av / nrm).astype(np.float32)


def kernel(x, conv_w, conv_b, centroids, att_w, att_b):
    x = np.asarray(x, np.float32)
    A, asum, _ = run_device(
        x, np.asarray(conv_w, np.float32), np.asarray(conv_b, np.float32)
    )
    return finalize(
        A, asum,
        np.asarray(centroids, np.float32),
        np.asarray(att_w, np.float32),
        np.asarray(att_b, np.float32),
    )


# revision 49
# speedup vs baseline: 1.0093x; 1.0093x over previous
"""AttVlad Trainium2 kernel.

Math (per image n):
  xn = x / ||x||_2(over d)                       x: [D=128, S]
  a  = softmax_k(conv_w @ xn + conv_b)           a: [K=64, S]
  vlad[k,d]   = sum_s a[k,s] xn[d,s] - (sum_s a[k,s]) centroids[k,d]
  out = normalize_d(vlad * (centroids @ att_w.T + att_b))

Device strategy (8 cores, data-parallel over n, 4 images each):
  - x is cast to bf16 on the host (the device math is bf16 either way, so
    this is bit-identical) and streamed in [128d, 4096s] HWDGE chunks,
    halving device HBM traffic.
  - Per 128-s unit: one PE pass with lhsT = x_chunk slice produces BOTH
    logits^T [128s, 64k] (rhs = conv_w^T) and x^T [128s, 128d] (rhs = I).
  - All per-s scalars (rsqrt of sumsq, softmax denom, their products) live
    as [128, 16] tiles (s on partitions) and are applied via broadcast
    (step-0) access patterns, so softmax needs no per-unit scalar ops.
  - Normalization scalars never touch x: logits are scaled by rnorm before
    exp; the VLAD matmul uses lhsT a2 = exp(l*rnorm)*exp(b) * (rnorm*rdenom)
    and rhs = [x^T | norm], giving columns [A | asum] accumulated in PSUM.
  - rsqrt is computed as exp(-0.5*ln(s)) to stay inside one ACT table set.
  - Host does the O(N*K*D) finalize (centroid subtract, attention scale,
    intra-normalization) in float64.
"""

import sys
import time

import numpy as np

try:  # the concourse stack (bass) ships in the container image
    import concourse.bass as _probe  # noqa: F401
except Exception:  # pragma: no cover
    sys.path.insert(0, "/opt/trn_rl_repo")

import ml_dtypes

N, D, S, K = 32, 128, 16384, 64
NCORES = 8
EPS = 1e-12

CHUNK = 4096  # s-positions per DMA chunk
UNIT = 128    # s-positions per matmul unit (psum partition dim)
XT_STRIDE = 130  # x^T unit stride in the SBUF tile: 128 cols x^T + 1 norm + 1 pad


def _make_tile_context_cls(tile, mybir, ScopedClock):
    """This walrus build rejects instructions carrying more than one sync
    wait; excess waits are split onto same-engine NoOps by _split_waits."""
    return tile.TileContext


# this walrus build rejects >1 sync wait on every instruction struct probed
# (CTRL, TT, MM); keep both caps at 1
MAX_WAITS = 1
COMPUTE_WAITS = 1
_COMPUTE_TYPES = (
    "InstTensorTensor", "InstActivation", "InstMatmult", "InstTensorReduce",
    "InstReciprocal", "InstTensorCopy", "InstLdweights", "InstTensorScalarPtr",
    "InstMemSet", "InstTensorScalar",
)


def _split_waits(nc, mybir):
    """Rewrite the traced BIR so no instruction carries more sem waits than
    this walrus build's per-struct limit: excess waits move to injected NoOps
    immediately preceding the instruction on the same engine (NX executes
    waits in order, so this is semantically identical)."""
    nid = 0
    for f in nc.m.functions:
        for blk in f.blocks:
            new_insts = []
            for inst in blk.instructions:
                si = getattr(inst, "sync_info", None)
                ws = list(si.on_wait) if si is not None else []
                maxw = (
                    COMPUTE_WAITS
                    if type(inst).__name__ in _COMPUTE_TYPES
                    else MAX_WAITS
                )
                if len(ws) > maxw:
                    extra = ws[: len(ws) - maxw]
                    for i in range(0, len(extra), MAX_WAITS):
                        nid += 1
                        nop = mybir.InstNoOp(
                            name=f"waitsplit_{nid}", ins=[], outs=[]
                        )
                        nop.engine = inst.engine
                        nop.sync_info = mybir.SyncInfo(
                            on_wait=extra[i : i + MAX_WAITS], on_update=[]
                        )
                        new_insts.append(nop)
                    si.on_wait = ws[len(ws) - maxw :]
                new_insts.append(inst)
            blk.instructions[:] = new_insts


# tunables (engine assignment variants, sweepable via the cost model)
OPT_SUMSQ = "pe"      # "dve": square+reduce on DVE | "pe": natural-layout square + ones-matmul
OPT_SOFT = "split"   # engine for a_un/a2 elementwise muls: "dve" | "gpsimd"


def build_program(n_per_core=4, s_total=S, reps=1, n_read=None):
    """Build the single-core Bass program (same program runs on all cores).
    reps>1 repeats the whole computation; n_read<n_per_core processes only
    the first n_read images (input shapes unchanged) — both are for
    slope-based HW timing."""
    if n_read is None:
        n_read = n_per_core
    import concourse.bass as bass
    import concourse.tile as tile
    from concourse import mybir
    from concourse.vector_clock import ScopedClock

    dt = mybir.dt
    AF = mybir.ActivationFunctionType
    OP = mybir.AluOpType

    TileContextFixed = _make_tile_context_cls(tile, mybir, ScopedClock)

    n_chunks = s_total // CHUNK
    units_per_chunk = CHUNK // UNIT
    HU = 8                                   # units per group (psum-bank sized)
    halves = tuple(range(units_per_chunk // HU))

    nc = bass.Bass()
    x_in = nc.declare_dram_parameter(
        "x", [n_per_core, D, s_total], dt.bfloat16, isOutput=False
    )
    wt_in = nc.declare_dram_parameter("wt", [D, K], dt.bfloat16, isOutput=False)
    idm_in = nc.declare_dram_parameter("idm", [D, D], dt.bfloat16, isOutput=False)
    bseed_in = nc.declare_dram_parameter(
        "bseed", [1, 2 * K], dt.float8e4, isOutput=False
    )
    ones_in = nc.declare_dram_parameter("ones", [D, 1], dt.bfloat16, isOutput=False)
    out_dram = nc.declare_dram_parameter(
        "out", [K, n_per_core * 132], dt.float32, isOutput=True
    )

    with TileContextFixed(nc) as tc:
        with (
            tc.tile_pool(name="consts", bufs=1) as consts,
            tc.tile_pool(name="xc", bufs=4) as xc_pool,
            tc.tile_pool(name="xt", bufs=4) as xt_pool,
            tc.tile_pool(name="soft", bufs=7) as soft_pool,
            tc.tile_pool(name="stats", bufs=4) as stats_pool,
            tc.tile_pool(name="scratch", bufs=3) as scratch_pool,
            tc.tile_pool(name="seed", bufs=3) as seed_pool,
            tc.tile_pool(name="outp", bufs=1) as out_pool,
            tc.tile_pool(name="psl", bufs=4, space="PSUM") as psl_pool,
            tc.tile_pool(name="pst", bufs=1, space="PSUM") as pst_pool,
            tc.tile_pool(name="pv", bufs=2, space="PSUM") as pv_pool,
            tc.tile_pool(name="pss", bufs=1, space="PSUM") as pss_pool,
        ):
            wt = consts.tile([D, K], dt.bfloat16)
            nc.sync.dma_start(wt[:], wt_in[:])
            idm = consts.tile([D, D], dt.bfloat16)
            nc.sync.dma_start(idm[:], idm_in[:])
            bseed = consts.tile([1, 2 * K], dt.float8e4)
            nc.sync.dma_start(bseed[:], bseed_in[:])
            bseed3 = bseed[:].rearrange("p (j n) -> p j n", j=2)
            ones = consts.tile([D, 1], dt.bfloat16)
            nc.sync.dma_start(ones[:], ones_in[:])

            out_sb = out_pool.tile([K, n_per_core * 132], dt.float32)
            # touch the ln/exp ACT table set immediately so its ~2.7us DMA
            # overlaps the initial input loads instead of the first chunk
            warm = consts.tile([1, 1], dt.float32)
            nc.scalar.activation(warm[:], ones[0:1, 0:1], AF.Ln)

            def emit_all():
              chunk_list = [
                  (n, ci) for n in range(n_read) for ci in range(n_chunks)
              ]
              lead_state = {}
              pv_state = {}

              def lead(n, ci):
                  """Per-chunk stats lead-in: load, square, per-unit sumsq
                  matmuls, rnorm. Emitted one chunk ahead of main() so the
                  baked in-order engine schedules interleave the next chunk's
                  lead-in with this chunk's softmax (no head-of-line block)."""
                  xc = xc_pool.tile([D, CHUNK], dt.bfloat16, name="xc")
                  # x is pre-cast to bf16 on the host (identical math to an
                  # on-device cast, half the HBM traffic). The very first
                  # chunk loads in quarters so compute starts ~2us sooner.
                  first_chunk = (n, ci) == (0, 0)
                  QC = CHUNK // 4
                  if first_chunk:
                      for q in range(4):
                          nc.gpsimd.dma_start(
                              xc[:, q * QC : (q + 1) * QC],
                              x_in[n, :, ci * CHUNK + q * QC
                                   : ci * CHUNK + (q + 1) * QC],
                          )
                  else:
                      nc.gpsimd.dma_start(
                          xc[:], x_in[n, :, ci * CHUNK : (ci + 1) * CHUNK]
                      )
                  rn = stats_pool.tile(
                      [128, units_per_chunk], dt.float32, tag="rn", name="rn"
                  )
                  lns = stats_pool.tile(
                      [128, units_per_chunk], dt.float32, tag="lns", name="lns"
                  )
                  # sumsq via PE: square x in natural layout, then per unit
                  # contract over d with a ones column, landing sumsq directly
                  # in s-partition orientation in PSUM.
                  xsq = scratch_pool.tile(
                      [D, CHUNK], dt.bfloat16, tag="xsq", name="xsq"
                  )
                  if first_chunk:
                      for q in range(4):
                          nc.vector.tensor_tensor(
                              out=xsq[:, q * QC : (q + 1) * QC],
                              in0=xc[:, q * QC : (q + 1) * QC],
                              in1=xc[:, q * QC : (q + 1) * QC], op=OP.mult,
                          )
                  elif ci % 4 == 3:
                      # balance: a quarter of the squares run on ACT
                      nc.scalar.activation(xsq[:], xc[:], AF.Square)
                  else:
                      nc.vector.tensor_tensor(
                          out=xsq[:], in0=xc[:], in1=xc[:], op=OP.mult
                      )
                  pss = pss_pool.tile([128, 96], dt.float32, name="pss")
                  ss = pss[:, 0:32]
                  for cu in range(units_per_chunk):
                      nc.tensor.matmul(
                          ss[:, cu : cu + 1],
                          xsq[:, cu * UNIT : (cu + 1) * UNIT],
                          ones[:], start=True, stop=True,
                      )
                  # rnorm = exp(-0.5*ln(sumsq)); stays inside one ACT table set
                  nc.scalar.activation(lns[:], ss[:], AF.Ln)
                  nc.scalar.activation(rn[:], lns[:], AF.Exp, scale=-0.5)
                  # one x^T tile per chunk; norm column (asum rhs) written now:
                  # norm = sqrt(sumsq) = exp(0.5*ln)
                  xt = xt_pool.tile(
                      [128, units_per_chunk * XT_STRIDE], dt.bfloat16, name="xt"
                  )
                  xt3 = xt[:].rearrange("p (u c) -> p u c", c=XT_STRIDE)
                  # fp8 seed row: norm = sqrt(ss) compact, PE-transposed to
                  # [32u, 128s], then partition-collapsed to one row so the
                  # per-unit rank-1 bias matmuls (norm[s]*b[k], DoubleRow)
                  # can use it as a base-partition-0 stationary
                  nr16 = stats_pool.tile(
                      [128, units_per_chunk], dt.bfloat16, tag="nr16",
                      name="nr16"
                  )
                  nc.scalar.activation(nr16[:], lns[:], AF.Exp, scale=0.5)
                  # asum column of xt copied from the compact norms on Pool
                  nc.gpsimd.tensor_copy(xt3[:, :, D : D + 1],
                                        nr16[:][:, :, None])
                  seedT = pss[0:32, 32:96].bitcast(dt.bfloat16)
                  nc.tensor.transpose(seedT, nr16[:], idm[:])
                  seed8 = seed_pool.tile(
                      [units_per_chunk, UNIT], dt.float8e4, tag="s8",
                      name="seed8"
                  )
                  nc.scalar.activation(seed8[:], seedT, AF.Copy)
                  seed8f = seed_pool.tile(
                      [1, units_per_chunk * UNIT], dt.float8e4, tag="s8f",
                      name="seed8f"
                  )
                  nc.sync.dma_start(seed8f[:], seed8[:])
                  lead_state[(n, ci)] = (xc, rn, xt, seed8f)

              def main(n, ci):
                  xc, rn, xt, seed8f = lead_state.pop((n, ci))
                  if ci == 0:
                      pv_state[n] = pv_pool.tile([K, 132], dt.float32, name="pv")
                  pv = pv_state[n]
                  xt3 = xt[:].rearrange("p (u c) -> p u c", c=XT_STRIDE)

                  psls = []
                  for h in halves:
                      psl = psl_pool.tile([128, HU * K], dt.float32, name="psl")
                      pst = pst_pool.tile([128, HU * D], dt.bfloat16, name="pst")
                      for u in range(HU):
                          cu = h * HU + u
                          lhsT = xc[:, cu * UNIT : (cu + 1) * UNIT]
                          srow = seed8f[0:1, cu * UNIT : (cu + 1) * UNIT][
                              :, None, :
                          ]
                          nc.tensor.matmul(
                              psl[:, u * K : (u + 1) * K],
                              srow.broadcast_to([1, 2, UNIT]), bseed3,
                              start=True, stop=False,
                              perf_mode=mybir.MatmulPerfMode.DoubleRow,
                          )
                          nc.tensor.matmul(
                              psl[:, u * K : (u + 1) * K], lhsT, wt[:],
                              start=False, stop=True,
                          )
                          nc.tensor.transpose(
                              pst[:, u * D : (u + 1) * D], lhsT, idm[:],
                          )
                      # batched PSUM->SBUF move of x^T (bf16), strided per unit
                      xt3h = xt3[:, h * HU : (h + 1) * HU, :]
                      pst3 = pst[:].rearrange("p (u c) -> p u c", c=D)
                      nc.scalar.activation(xt3h[:, :, 0:D], pst3, AF.Copy)
                      psls.append(psl)

                  for h in halves:
                      psl = psls[h]
                      rnh = rn[:, h * HU : (h + 1) * HU]
                      # l_scaled = logits_raw * rnorm (broadcast over k)
                      lsc = soft_pool.tile(
                          [128, HU * K], dt.bfloat16, tag="lsc", name="lsc"
                      )
                      nc.vector.tensor_tensor(
                          out=lsc[:].rearrange("p (u k) -> p u k", k=K),
                          in0=psl[:].rearrange("p (u k) -> p u k", k=K),
                          in1=rnh.broadcast_to([128, HU, K]),
                          op=OP.mult,
                      )
                      e = soft_pool.tile(
                          [128, HU * K], dt.bfloat16, tag="e", name="e"
                      )
                      nc.scalar.activation(e[:], lsc[:], AF.Exp)
                      # bias already folded in via the PE seed: e includes
                      # exp(b), so the denominator reduces e directly
                      dn = stats_pool.tile([128, HU], dt.float32, tag="dn", name="dn")
                      nc.vector.tensor_reduce(
                          out=dn[:],
                          in_=e[:].rearrange("p (u k) -> p u k", k=K),
                          axis=mybir.AxisListType.X, op=OP.add,
                      )
                      rdn = stats_pool.tile(
                          [128, HU], dt.float32, tag="rdn", name="rdn"
                      )
                      nc.vector.reciprocal(rdn[:], dn[:])
                      cc = stats_pool.tile([128, HU], dt.float32, tag="cc", name="cc")
                      nc.gpsimd.tensor_tensor(
                          out=cc[:], in0=rnh, in1=rdn[:], op=OP.mult
                      )
                      # a2 = a_un * (rnorm * rdenom)
                      a2 = soft_pool.tile(
                          [128, HU * K], dt.bfloat16, tag="a2", name="a2"
                      )
                      if OPT_SOFT == "gpsimd_ccb" or (
                          OPT_SOFT == "split" and h % 2 == 1
                      ):
                          # materialize cc broadcast (gpsimd 1-input) so the
                          # a2 multiply runs in the DVE 2x bf16 mode
                          ccb = soft_pool.tile(
                              [128, HU * K], dt.bfloat16, tag="ccb", name="ccb"
                          )
                          nc.gpsimd.tensor_copy(
                              ccb[:].rearrange("p (u k) -> p u k", k=K),
                              cc[:].broadcast_to([128, HU, K]),
                          )
                          nc.vector.tensor_tensor(
                              out=a2[:], in0=e[:], in1=ccb[:], op=OP.mult
                          )
                      else:
                          nc.vector.tensor_tensor(
                              out=a2[:].rearrange("p (u k) -> p u k", k=K),
                              in0=e[:].rearrange("p (u k) -> p u k", k=K),
                              in1=cc[:].broadcast_to([128, HU, K]),
                              op=OP.mult,
                          )
                      # VLAD accumulation: pv[:, :129] += a2_u^T @ [x^T | norm]
                      for u in range(HU):
                          cu = ci * units_per_chunk + h * HU + u
                          first = cu == 0
                          last = cu == (s_total // UNIT) - 1
                          xoff = (h * HU + u) * XT_STRIDE
                          nc.tensor.matmul(
                              pv[:, 0 : D + 1],
                              a2[:, u * K : (u + 1) * K],
                              xt[:, xoff : xoff + D + 1],
                              start=first, stop=last,
                          )
                  if ci == n_chunks - 1:
                      # stash [A | asum] for this n and ship it immediately
                      # so only the last image's store sits in the tail
                      nc.scalar.activation(
                          out_sb[:, n * 132 : n * 132 + D + 1],
                          pv[:, 0 : D + 1], AF.Copy,
                      )
                      nc.sync.dma_start(
                          out_dram[:, n * 132 : n * 132 + D + 1],
                          out_sb[:, n * 132 : n * 132 + D + 1],
                      )

              lead(*chunk_list[0])
              for i, (n, ci) in enumerate(chunk_list):
                  if i + 1 < len(chunk_list):
                      lead(*chunk_list[i + 1])
                  main(n, ci)
            if reps > 1:
                with tc.For_i(0, reps, 1):
                    emit_all()
            else:
                emit_all()

    _split_waits(nc, mybir)
    return nc


_CACHE = {}


def _get_program(n_per_core, s_total, reps=1, n_read=None):
    key = (n_per_core, s_total, reps, n_read)
    if key not in _CACHE:
        _CACHE[key] = build_program(n_per_core, s_total, reps, n_read)
    return _CACHE[key]


def run_device(x, conv_w, conv_b, n_per_core=4, s_total=S, trace=False):
    """Run the device part. x: [NCORES*n_per_core, D, s_total] fp32.
    Returns (A [n, K, D], asum [n, K], bass_results)."""
    from concourse.bass_utils import run_bass_kernel_spmd

    nc = _get_program(n_per_core, s_total)

    bf16 = ml_dtypes.bfloat16
    f8 = ml_dtypes.float8_e4m3
    wt_np = np.ascontiguousarray(conv_w.T.astype(bf16))           # [D, K]
    idm_np = np.eye(D, dtype=bf16)                                 # [D, D]
    bseed_np = np.concatenate(
        [conv_b.astype(np.float32), np.zeros(K, np.float32)]
    ).reshape(1, 2 * K).astype(f8)

    ones_np = np.ones((D, 1), bf16)
    in_maps = []
    for c in range(NCORES):
        xc = np.ascontiguousarray(
            x[c * n_per_core : (c + 1) * n_per_core].astype(bf16)
        )
        in_maps.append(
            {"x": xc, "wt": wt_np, "idm": idm_np, "bseed": bseed_np,
             "ones": ones_np}
        )

    try:
        res = run_bass_kernel_spmd(
            nc, in_maps, list(range(NCORES)), trace=trace,
        )
    except Exception:
        # one retry: the device occasionally reports a transient
        # unrecoverable state right after a failed prior load
        time.sleep(2)
        res = run_bass_kernel_spmd(
            nc, in_maps, list(range(NCORES)), trace=trace,
        )

    n_total = NCORES * n_per_core
    A = np.empty((n_total, K, D), np.float64)
    asum = np.empty((n_total, K), np.float64)
    for c in range(NCORES):
        o = res.results[c]["out"]  # [K, n_per_core*132]
        for nl in range(n_per_core):
            blk = o[:, nl * 132 : nl * 132 + D + 1].astype(np.float64)
            A[c * n_per_core + nl] = blk[:, :D]
            asum[c * n_per_core + nl] = blk[:, D]
    return A, asum, res


def finalize(A, asum, centroids, att_w, att_b):
    cen = centroids.astype(np.float64)
    vlad = A - asum[:, :, None] * cen[None]
    soft = cen @ att_w.astype(np.float64).T + att_b.astype(np.float64)  # [K, 1]
    av = vlad * soft[None]
    nrm = np.maximum(np.linalg.norm(av, axis=2, keepdims=True), EPS)
    return (av / nrm).astype(np.float32)


def kernel(x, conv_w, conv_b, centroids, att_w, att_b):
    x = np.asarray(x, np.float32)
    A, asum, _ = run_device(
        x, np.asarray(conv_w, np.float32), np.asarray(conv_b, np.float32)
    )
    return finalize(
        A, asum,
        np.asarray(centroids, np.float32),
        np.asarray(att_w, np.float32),
        np.asarray(att_b, np.float32),
    )


# revision 52
# speedup vs baseline: 1.0272x; 1.0177x over previous
"""AttVlad Trainium2 kernel.

Math (per image n):
  xn = x / ||x||_2(over d)                       x: [D=128, S]
  a  = softmax_k(conv_w @ xn + conv_b)           a: [K=64, S]
  vlad[k,d]   = sum_s a[k,s] xn[d,s] - (sum_s a[k,s]) centroids[k,d]
  out = normalize_d(vlad * (centroids @ att_w.T + att_b))

Device strategy (8 cores, data-parallel over n, 4 images each):
  - x is cast to bf16 on the host (the device math is bf16 either way, so
    this is bit-identical) and streamed in [128d, 4096s] HWDGE chunks,
    halving device HBM traffic.
  - Per 128-s unit: one PE pass with lhsT = x_chunk slice produces BOTH
    logits^T [128s, 64k] (rhs = conv_w^T) and x^T [128s, 128d] (rhs = I).
  - All per-s scalars (rsqrt of sumsq, softmax denom, their products) live
    as [128, 16] tiles (s on partitions) and are applied via broadcast
    (step-0) access patterns, so softmax needs no per-unit scalar ops.
  - Normalization scalars never touch x: logits are scaled by rnorm before
    exp; the VLAD matmul uses lhsT a2 = exp(l*rnorm)*exp(b) * (rnorm*rdenom)
    and rhs = [x^T | norm], giving columns [A | asum] accumulated in PSUM.
  - rsqrt is computed as exp(-0.5*ln(s)) to stay inside one ACT table set.
  - Host does the O(N*K*D) finalize (centroid subtract, attention scale,
    intra-normalization) in float64.
"""

import sys
import time

import numpy as np

try:  # the concourse stack (bass) ships in the container image
    import concourse.bass as _probe  # noqa: F401
except Exception:  # pragma: no cover
    sys.path.insert(0, "/opt/trn_rl_repo")

import ml_dtypes

N, D, S, K = 32, 128, 16384, 64
NCORES = 8
EPS = 1e-12

CHUNK = 4096  # s-positions per DMA chunk
UNIT = 128    # s-positions per matmul unit (psum partition dim)
XT_STRIDE = 130  # x^T unit stride in the SBUF tile: 128 cols x^T + 1 norm + 1 pad


def _make_tile_context_cls(tile, mybir, ScopedClock):
    """This walrus build rejects instructions carrying more than one sync
    wait; excess waits are split onto same-engine NoOps by _split_waits."""
    return tile.TileContext


# this walrus build rejects >1 sync wait on every instruction struct probed
# (CTRL, TT, MM); keep both caps at 1
MAX_WAITS = 1
COMPUTE_WAITS = 1
_COMPUTE_TYPES = (
    "InstTensorTensor", "InstActivation", "InstMatmult", "InstTensorReduce",
    "InstReciprocal", "InstTensorCopy", "InstLdweights", "InstTensorScalarPtr",
    "InstMemSet", "InstTensorScalar",
)


def _split_waits(nc, mybir):
    """Rewrite the traced BIR so no instruction carries more sem waits than
    this walrus build's per-struct limit: excess waits move to injected NoOps
    immediately preceding the instruction on the same engine (NX executes
    waits in order, so this is semantically identical)."""
    nid = 0
    for f in nc.m.functions:
        for blk in f.blocks:
            new_insts = []
            for inst in blk.instructions:
                si = getattr(inst, "sync_info", None)
                ws = list(si.on_wait) if si is not None else []
                maxw = (
                    COMPUTE_WAITS
                    if type(inst).__name__ in _COMPUTE_TYPES
                    else MAX_WAITS
                )
                if len(ws) > maxw:
                    extra = ws[: len(ws) - maxw]
                    for i in range(0, len(extra), MAX_WAITS):
                        nid += 1
                        nop = mybir.InstNoOp(
                            name=f"waitsplit_{nid}", ins=[], outs=[]
                        )
                        nop.engine = inst.engine
                        nop.sync_info = mybir.SyncInfo(
                            on_wait=extra[i : i + MAX_WAITS], on_update=[]
                        )
                        new_insts.append(nop)
                    si.on_wait = ws[len(ws) - maxw :]
                new_insts.append(inst)
            blk.instructions[:] = new_insts


# tunables (engine assignment variants, sweepable via the cost model)
OPT_SUMSQ = "pe"      # "dve": square+reduce on DVE | "pe": natural-layout square + ones-matmul
OPT_SOFT = "split"   # engine for a_un/a2 elementwise muls: "dve" | "gpsimd"


def build_program(n_per_core=4, s_total=S, reps=1, n_read=None):
    """Build the single-core Bass program (same program runs on all cores).
    reps>1 repeats the whole computation; n_read<n_per_core processes only
    the first n_read images (input shapes unchanged) — both are for
    slope-based HW timing."""
    if n_read is None:
        n_read = n_per_core
    import concourse.bass as bass
    import concourse.tile as tile
    from concourse import mybir
    from concourse.vector_clock import ScopedClock

    dt = mybir.dt
    AF = mybir.ActivationFunctionType
    OP = mybir.AluOpType

    TileContextFixed = _make_tile_context_cls(tile, mybir, ScopedClock)

    n_chunks = s_total // CHUNK
    units_per_chunk = CHUNK // UNIT
    HU = 8                                   # units per group (psum-bank sized)
    halves = tuple(range(units_per_chunk // HU))

    nc = bass.Bass()
    x_in = nc.declare_dram_parameter(
        "x", [n_per_core, D, s_total], dt.bfloat16, isOutput=False
    )
    wt_in = nc.declare_dram_parameter("wt", [D, K], dt.bfloat16, isOutput=False)
    idm_in = nc.declare_dram_parameter("idm", [D, D], dt.bfloat16, isOutput=False)
    bseed_in = nc.declare_dram_parameter(
        "bseed", [1, 2 * K], dt.float8e4, isOutput=False
    )
    ones_in = nc.declare_dram_parameter("ones", [D, 1], dt.bfloat16, isOutput=False)
    out_dram = nc.declare_dram_parameter(
        "out", [K, n_per_core * 132], dt.float32, isOutput=True
    )

    with TileContextFixed(nc) as tc:
        with (
            tc.tile_pool(name="consts", bufs=1) as consts,
            tc.tile_pool(name="xc", bufs=5) as xc_pool,
            tc.tile_pool(name="xt", bufs=5) as xt_pool,
            tc.tile_pool(name="soft", bufs=9) as soft_pool,
            tc.tile_pool(name="stats", bufs=6) as stats_pool,
            tc.tile_pool(name="scratch", bufs=4) as scratch_pool,
            tc.tile_pool(name="seed", bufs=4) as seed_pool,
            tc.tile_pool(name="outp", bufs=1) as out_pool,
            tc.tile_pool(name="psl", bufs=4, space="PSUM") as psl_pool,
            tc.tile_pool(name="pst", bufs=2, space="PSUM") as pst_pool,
            tc.tile_pool(name="pv", bufs=1, space="PSUM") as pv_pool,
            tc.tile_pool(name="pss", bufs=1, space="PSUM") as pss_pool,
        ):
            wt = consts.tile([D, K], dt.bfloat16)
            nc.sync.dma_start(wt[:], wt_in[:])
            idm = consts.tile([D, D], dt.bfloat16)
            nc.sync.dma_start(idm[:], idm_in[:])
            bseed = consts.tile([1, 2 * K], dt.float8e4)
            nc.sync.dma_start(bseed[:], bseed_in[:])
            bseed3 = bseed[:].rearrange("p (j n) -> p j n", j=2)
            ones = consts.tile([D, 1], dt.bfloat16)
            nc.sync.dma_start(ones[:], ones_in[:])

            out_sb = out_pool.tile([K, n_per_core * 132], dt.float32)
            # touch the ln/exp ACT table set immediately so its ~2.7us DMA
            # overlaps the initial input loads instead of the first chunk
            warm = consts.tile([1, 1], dt.float32)
            nc.scalar.activation(warm[:], ones[0:1, 0:1], AF.Ln)

            def emit_all():
              chunk_list = [
                  (n, ci) for n in range(n_read) for ci in range(n_chunks)
              ]
              lead_state = {}
              pv_state = {}

              def lead(n, ci):
                  """Per-chunk stats lead-in: load, square, per-unit sumsq
                  matmuls, rnorm. Emitted one chunk ahead of main() so the
                  baked in-order engine schedules interleave the next chunk's
                  lead-in with this chunk's softmax (no head-of-line block)."""
                  xc = xc_pool.tile([D, CHUNK], dt.bfloat16, name="xc")
                  # x is pre-cast to bf16 on the host (identical math to an
                  # on-device cast, half the HBM traffic). The very first
                  # chunk loads in quarters so compute starts ~2us sooner.
                  first_chunk = (n, ci) == (0, 0)
                  QC = CHUNK // 4
                  if first_chunk:
                      for q in range(4):
                          nc.gpsimd.dma_start(
                              xc[:, q * QC : (q + 1) * QC],
                              x_in[n, :, ci * CHUNK + q * QC
                                   : ci * CHUNK + (q + 1) * QC],
                          )
                  else:
                      nc.gpsimd.dma_start(
                          xc[:], x_in[n, :, ci * CHUNK : (ci + 1) * CHUNK]
                      )
                  rn = stats_pool.tile(
                      [128, units_per_chunk], dt.float32, tag="rn", name="rn"
                  )
                  lns = stats_pool.tile(
                      [128, units_per_chunk], dt.float32, tag="lns", name="lns"
                  )
                  # sumsq via PE: square x in natural layout, then per unit
                  # contract over d with a ones column, landing sumsq directly
                  # in s-partition orientation in PSUM.
                  xsq = scratch_pool.tile(
                      [D, CHUNK], dt.bfloat16, tag="xsq", name="xsq"
                  )
                  if first_chunk:
                      for q in range(4):
                          nc.vector.tensor_tensor(
                              out=xsq[:, q * QC : (q + 1) * QC],
                              in0=xc[:, q * QC : (q + 1) * QC],
                              in1=xc[:, q * QC : (q + 1) * QC], op=OP.mult,
                          )
                  elif ci % 4 == 3:
                      # balance: a quarter of the squares run on ACT
                      nc.scalar.activation(xsq[:], xc[:], AF.Square)
                  else:
                      nc.vector.tensor_tensor(
                          out=xsq[:], in0=xc[:], in1=xc[:], op=OP.mult
                      )
                  pss = pss_pool.tile([128, 96], dt.float32, name="pss")
                  ss = pss[:, 0:32]
                  for cu in range(units_per_chunk):
                      nc.tensor.matmul(
                          ss[:, cu : cu + 1],
                          xsq[:, cu * UNIT : (cu + 1) * UNIT],
                          ones[:], start=True, stop=True,
                      )
                  # rnorm = exp(-0.5*ln(sumsq)); stays inside one ACT table set
                  nc.scalar.activation(lns[:], ss[:], AF.Ln)
                  nc.scalar.activation(rn[:], lns[:], AF.Exp, scale=-0.5)
                  # one x^T tile per chunk; norm column (asum rhs) written now:
                  # norm = sqrt(sumsq) = exp(0.5*ln)
                  xt = xt_pool.tile(
                      [128, units_per_chunk * XT_STRIDE], dt.bfloat16, name="xt"
                  )
                  xt3 = xt[:].rearrange("p (u c) -> p u c", c=XT_STRIDE)
                  # fp8 seed row: norm = sqrt(ss) compact, PE-transposed to
                  # [32u, 128s], then partition-collapsed to one row so the
                  # per-unit rank-1 bias matmuls (norm[s]*b[k], DoubleRow)
                  # can use it as a base-partition-0 stationary
                  nr16 = stats_pool.tile(
                      [128, units_per_chunk], dt.bfloat16, tag="nr16",
                      name="nr16"
                  )
                  nc.scalar.activation(nr16[:], lns[:], AF.Exp, scale=0.5)
                  # asum column of xt copied from the compact norms on Pool
                  nc.gpsimd.tensor_copy(xt3[:, :, D : D + 1],
                                        nr16[:][:, :, None])
                  seedT = pss[0:32, 32:96].bitcast(dt.bfloat16)
                  nc.tensor.transpose(seedT, nr16[:], idm[:])
                  seed8 = seed_pool.tile(
                      [units_per_chunk, UNIT], dt.float8e4, tag="s8",
                      name="seed8"
                  )
                  nc.scalar.activation(seed8[:], seedT, AF.Copy)
                  seed8f = seed_pool.tile(
                      [1, units_per_chunk * UNIT], dt.float8e4, tag="s8f",
                      name="seed8f"
                  )
                  nc.sync.dma_start(seed8f[:], seed8[:])
                  lead_state[(n, ci)] = (xc, rn, xt, seed8f)

              def main(n, ci):
                  xc, rn, xt, seed8f = lead_state.pop((n, ci))
                  if ci == 0:
                      pv_state[n] = pv_pool.tile([K, 132], dt.float32, name="pv")
                  pv = pv_state[n]
                  xt3 = xt[:].rearrange("p (u c) -> p u c", c=XT_STRIDE)

                  psls = []
                  for h in halves:
                      psl = psl_pool.tile([128, HU * K], dt.float32, name="psl")
                      pst = pst_pool.tile([128, HU * D], dt.bfloat16, name="pst")
                      for u in range(HU):
                          cu = h * HU + u
                          lhsT = xc[:, cu * UNIT : (cu + 1) * UNIT]
                          srow = seed8f[0:1, cu * UNIT : (cu + 1) * UNIT][
                              :, None, :
                          ]
                          nc.tensor.matmul(
                              psl[:, u * K : (u + 1) * K],
                              srow.broadcast_to([1, 2, UNIT]), bseed3,
                              start=True, stop=False,
                              perf_mode=mybir.MatmulPerfMode.DoubleRow,
                          )
                          nc.tensor.matmul(
                              psl[:, u * K : (u + 1) * K], lhsT, wt[:],
                              start=False, stop=True,
                          )
                          nc.tensor.transpose(
                              pst[:, u * D : (u + 1) * D], lhsT, idm[:],
                          )
                      # batched PSUM->SBUF move of x^T (bf16), strided per unit
                      xt3h = xt3[:, h * HU : (h + 1) * HU, :]
                      pst3 = pst[:].rearrange("p (u c) -> p u c", c=D)
                      nc.scalar.activation(xt3h[:, :, 0:D], pst3, AF.Copy)
                      psls.append(psl)

                  for h in halves:
                      psl = psls[h]
                      rnh = rn[:, h * HU : (h + 1) * HU]
                      # l_scaled = logits_raw * rnorm (broadcast over k)
                      lsc = soft_pool.tile(
                          [128, HU * K], dt.bfloat16, tag="lsc", name="lsc"
                      )
                      nc.vector.tensor_tensor(
                          out=lsc[:].rearrange("p (u k) -> p u k", k=K),
                          in0=psl[:].rearrange("p (u k) -> p u k", k=K),
                          in1=rnh.broadcast_to([128, HU, K]),
                          op=OP.mult,
                      )
                      e = soft_pool.tile(
                          [128, HU * K], dt.bfloat16, tag="e", name="e"
                      )
                      nc.scalar.activation(e[:], lsc[:], AF.Exp)
                      # bias already folded in via the PE seed: e includes
                      # exp(b), so the denominator reduces e directly
                      dn = stats_pool.tile([128, HU], dt.float32, tag="dn", name="dn")
                      nc.vector.tensor_reduce(
                          out=dn[:],
                          in_=e[:].rearrange("p (u k) -> p u k", k=K),
                          axis=mybir.AxisListType.X, op=OP.add,
                      )
                      rdn = stats_pool.tile(
                          [128, HU], dt.float32, tag="rdn", name="rdn"
                      )
                      nc.vector.reciprocal(rdn[:], dn[:])
                      cc = stats_pool.tile([128, HU], dt.float32, tag="cc", name="cc")
                      nc.gpsimd.tensor_tensor(
                          out=cc[:], in0=rnh, in1=rdn[:], op=OP.mult
                      )
                      # a2 = a_un * (rnorm * rdenom)
                      a2 = soft_pool.tile(
                          [128, HU * K], dt.bfloat16, tag="a2", name="a2"
                      )
                      if OPT_SOFT == "gpsimd_ccb" or (
                          OPT_SOFT == "split" and h % 2 == 1
                      ):
                          # materialize cc broadcast (gpsimd 1-input) so the
                          # a2 multiply runs in the DVE 2x bf16 mode
                          ccb = soft_pool.tile(
                              [128, HU * K], dt.bfloat16, tag="ccb", name="ccb"
                          )
                          nc.gpsimd.tensor_copy(
                              ccb[:].rearrange("p (u k) -> p u k", k=K),
                              cc[:].broadcast_to([128, HU, K]),
                          )
                          nc.vector.tensor_tensor(
                              out=a2[:], in0=e[:], in1=ccb[:], op=OP.mult
                          )
                      else:
                          nc.vector.tensor_tensor(
                              out=a2[:].rearrange("p (u k) -> p u k", k=K),
                              in0=e[:].rearrange("p (u k) -> p u k", k=K),
                              in1=cc[:].broadcast_to([128, HU, K]),
                              op=OP.mult,
                          )
                      # VLAD accumulation: pv[:, :129] += a2_u^T @ [x^T | norm]
                      for u in range(HU):
                          cu = ci * units_per_chunk + h * HU + u
                          first = cu == 0
                          last = cu == (s_total // UNIT) - 1
                          xoff = (h * HU + u) * XT_STRIDE
                          nc.tensor.matmul(
                              pv[:, 0 : D + 1],
                              a2[:, u * K : (u + 1) * K],
                              xt[:, xoff : xoff + D + 1],
                              start=first, stop=last,
                          )
                  if ci == n_chunks - 1:
                      # stash [A | asum] for this n and ship it immediately
                      # so only the last image's store sits in the tail
                      nc.scalar.activation(
                          out_sb[:, n * 132 : n * 132 + D + 1],
                          pv[:, 0 : D + 1], AF.Copy,
                      )
                      nc.sync.dma_start(
                          out_dram[:, n * 132 : n * 132 + D + 1],
                          out_sb[:, n * 132 : n * 132 + D + 1],
                      )

              lead(*chunk_list[0])
              for i, (n, ci) in enumerate(chunk_list):
                  if i + 1 < len(chunk_list):
                      lead(*chunk_list[i + 1])
                  main(n, ci)
            if reps > 1:
                with tc.For_i(0, reps, 1):
                    emit_all()
            else:
                emit_all()

    _split_waits(nc, mybir)
    return nc


_CACHE = {}


def _get_program(n_per_core, s_total, reps=1, n_read=None):
    key = (n_per_core, s_total, reps, n_read)
    if key not in _CACHE:
        _CACHE[key] = build_program(n_per_core, s_total, reps, n_read)
    return _CACHE[key]


def run_device(x, conv_w, conv_b, n_per_core=4, s_total=S, trace=False):
    """Run the device part. x: [NCORES*n_per_core, D, s_total] fp32.
    Returns (A [n, K, D], asum [n, K], bass_results)."""
    from concourse.bass_utils import run_bass_kernel_spmd

    nc = _get_program(n_per_core, s_total)

    bf16 = ml_dtypes.bfloat16
    f8 = ml_dtypes.float8_e4m3
    wt_np = np.ascontiguousarray(conv_w.T.astype(bf16))           # [D, K]
    idm_np = np.eye(D, dtype=bf16)                                 # [D, D]
    bseed_np = np.concatenate(
        [conv_b.astype(np.float32), np.zeros(K, np.float32)]
    ).reshape(1, 2 * K).astype(f8)

    ones_np = np.ones((D, 1), bf16)
    in_maps = []
    for c in range(NCORES):
        xc = np.ascontiguousarray(
            x[c * n_per_core : (c + 1) * n_per_core].astype(bf16)
        )
        in_maps.append(
            {"x": xc, "wt": wt_np, "idm": idm_np, "bseed": bseed_np,
             "ones": ones_np}
        )

    try:
        res = run_bass_kernel_spmd(
            nc, in_maps, list(range(NCORES)), trace=trace,
        )
    except Exception:
        # one retry: the device occasionally reports a transient
        # unrecoverable state right after a failed prior load
        time.sleep(2)
        res = run_bass_kernel_spmd(
            nc, in_maps, list(range(NCORES)), trace=trace,
        )

    n_total = NCORES * n_per_core
    A = np.empty((n_total, K, D), np.float64)
    asum = np.empty((n_total, K), np.float64)
    for c in range(NCORES):
        o = res.results[c]["out"]  # [K, n_per_core*132]
        for nl in range(n_per_core):
            blk = o[:, nl * 132 : nl * 132 + D + 1].astype(np.float64)
            A[c * n_per_core + nl] = blk[:, :D]
            asum[c * n_per_core + nl] = blk[:, D]
    return A, asum, res


def finalize(A, asum, centroids, att_w, att_b):
    cen = centroids.astype(np.float64)
    vlad = A - asum[:, :, None] * cen[None]
    soft = cen @ att_w.astype(np.float64).T + att_b.astype(np.float64)  # [K, 1]
    av = vlad * soft[None]
    nrm = np.maximum(np.linalg.norm(av, axis=2, keepdims=True), EPS)
    return (av / nrm).astype(np.float32)


def kernel(x, conv_w, conv_b, centroids, att_w, att_b):
    x = np.asarray(x, np.float32)
    A, asum, _ = run_device(
        x, np.asarray(conv_w, np.float32), np.asarray(conv_b, np.float32)
    )
    return finalize(
        A, asum,
        np.asarray(centroids, np.float32),
        np.asarray(att_w, np.float32),
        np.asarray(att_b, np.float32),
    )


# revision 53
# speedup vs baseline: 1.0384x; 1.0109x over previous
"""AttVlad Trainium2 kernel.

Math (per image n):
  xn = x / ||x||_2(over d)                       x: [D=128, S]
  a  = softmax_k(conv_w @ xn + conv_b)           a: [K=64, S]
  vlad[k,d]   = sum_s a[k,s] xn[d,s] - (sum_s a[k,s]) centroids[k,d]
  out = normalize_d(vlad * (centroids @ att_w.T + att_b))

Device strategy (8 cores, data-parallel over n, 4 images each):
  - x is cast to bf16 on the host (the device math is bf16 either way, so
    this is bit-identical) and streamed in [128d, 4096s] HWDGE chunks,
    halving device HBM traffic.
  - Per 128-s unit: one PE pass with lhsT = x_chunk slice produces BOTH
    logits^T [128s, 64k] (rhs = conv_w^T) and x^T [128s, 128d] (rhs = I).
  - All per-s scalars (rsqrt of sumsq, softmax denom, their products) live
    as [128, 16] tiles (s on partitions) and are applied via broadcast
    (step-0) access patterns, so softmax needs no per-unit scalar ops.
  - Normalization scalars never touch x: logits are scaled by rnorm before
    exp; the VLAD matmul uses lhsT a2 = exp(l*rnorm)*exp(b) * (rnorm*rdenom)
    and rhs = [x^T | norm], giving columns [A | asum] accumulated in PSUM.
  - rsqrt is computed as exp(-0.5*ln(s)) to stay inside one ACT table set.
  - Host does the O(N*K*D) finalize (centroid subtract, attention scale,
    intra-normalization) in float64.
"""

import sys
import time

import numpy as np

try:  # the concourse stack (bass) ships in the container image
    import concourse.bass as _probe  # noqa: F401
except Exception:  # pragma: no cover
    sys.path.insert(0, "/opt/trn_rl_repo")

import ml_dtypes

N, D, S, K = 32, 128, 16384, 64
NCORES = 8
EPS = 1e-12

CHUNK = 4096  # s-positions per DMA chunk
UNIT = 128    # s-positions per matmul unit (psum partition dim)
XT_STRIDE = 130  # x^T unit stride in the SBUF tile: 128 cols x^T + 1 norm + 1 pad


def _make_tile_context_cls(tile, mybir, ScopedClock):
    """This walrus build rejects instructions carrying more than one sync
    wait; excess waits are split onto same-engine NoOps by _split_waits."""
    return tile.TileContext


# this walrus build rejects >1 sync wait on every instruction struct probed
# (CTRL, TT, MM); keep both caps at 1
MAX_WAITS = 1
COMPUTE_WAITS = 1
_COMPUTE_TYPES = (
    "InstTensorTensor", "InstActivation", "InstMatmult", "InstTensorReduce",
    "InstReciprocal", "InstTensorCopy", "InstLdweights", "InstTensorScalarPtr",
    "InstMemSet", "InstTensorScalar",
)


def _split_waits(nc, mybir):
    """Rewrite the traced BIR so no instruction carries more sem waits than
    this walrus build's per-struct limit: excess waits move to injected NoOps
    immediately preceding the instruction on the same engine (NX executes
    waits in order, so this is semantically identical)."""
    nid = 0
    for f in nc.m.functions:
        for blk in f.blocks:
            new_insts = []
            for inst in blk.instructions:
                si = getattr(inst, "sync_info", None)
                ws = list(si.on_wait) if si is not None else []
                maxw = (
                    COMPUTE_WAITS
                    if type(inst).__name__ in _COMPUTE_TYPES
                    else MAX_WAITS
                )
                if len(ws) > maxw:
                    extra = ws[: len(ws) - maxw]
                    for i in range(0, len(extra), MAX_WAITS):
                        nid += 1
                        nop = mybir.InstNoOp(
                            name=f"waitsplit_{nid}", ins=[], outs=[]
                        )
                        nop.engine = inst.engine
                        nop.sync_info = mybir.SyncInfo(
                            on_wait=extra[i : i + MAX_WAITS], on_update=[]
                        )
                        new_insts.append(nop)
                    si.on_wait = ws[len(ws) - maxw :]
                new_insts.append(inst)
            blk.instructions[:] = new_insts


# tunables (engine assignment variants, sweepable via the cost model)
OPT_SUMSQ = "pe"      # "dve": square+reduce on DVE | "pe": natural-layout square + ones-matmul
OPT_SOFT = "split"   # engine for a_un/a2 elementwise muls: "dve" | "gpsimd"


def build_program(n_per_core=4, s_total=S, reps=1, n_read=None):
    """Build the single-core Bass program (same program runs on all cores).
    reps>1 repeats the whole computation; n_read<n_per_core processes only
    the first n_read images (input shapes unchanged) — both are for
    slope-based HW timing."""
    if n_read is None:
        n_read = n_per_core
    import concourse.bass as bass
    import concourse.tile as tile
    from concourse import mybir
    from concourse.vector_clock import ScopedClock

    dt = mybir.dt
    AF = mybir.ActivationFunctionType
    OP = mybir.AluOpType

    TileContextFixed = _make_tile_context_cls(tile, mybir, ScopedClock)

    n_chunks = s_total // CHUNK
    units_per_chunk = CHUNK // UNIT
    HU = 8                                   # units per group (psum-bank sized)
    halves = tuple(range(units_per_chunk // HU))

    nc = bass.Bass()
    x_in = nc.declare_dram_parameter(
        "x", [n_per_core, D, s_total], dt.bfloat16, isOutput=False
    )
    wt_in = nc.declare_dram_parameter("wt", [D, K], dt.bfloat16, isOutput=False)
    idm_in = nc.declare_dram_parameter("idm", [D, D], dt.bfloat16, isOutput=False)
    bseed_in = nc.declare_dram_parameter(
        "bseed", [1, 2 * K], dt.float8e4, isOutput=False
    )
    ones_in = nc.declare_dram_parameter("ones", [D, 1], dt.bfloat16, isOutput=False)
    out_dram = nc.declare_dram_parameter(
        "out", [K, n_per_core * 132], dt.float32, isOutput=True
    )

    with TileContextFixed(nc) as tc:
        with (
            tc.tile_pool(name="consts", bufs=1) as consts,
            tc.tile_pool(name="xc", bufs=5) as xc_pool,
            tc.tile_pool(name="xt", bufs=5) as xt_pool,
            tc.tile_pool(name="soft", bufs=9) as soft_pool,
            tc.tile_pool(name="stats", bufs=6) as stats_pool,
            tc.tile_pool(name="scratch", bufs=4) as scratch_pool,
            tc.tile_pool(name="seed", bufs=4) as seed_pool,
            tc.tile_pool(name="outp", bufs=1) as out_pool,
            tc.tile_pool(name="psl", bufs=4, space="PSUM") as psl_pool,
            tc.tile_pool(name="pst", bufs=2, space="PSUM") as pst_pool,
            tc.tile_pool(name="pv", bufs=1, space="PSUM") as pv_pool,
            tc.tile_pool(name="pss", bufs=1, space="PSUM") as pss_pool,
        ):
            wt = consts.tile([D, K], dt.bfloat16)
            nc.sync.dma_start(wt[:], wt_in[:])
            idm = consts.tile([D, D], dt.bfloat16)
            nc.sync.dma_start(idm[:], idm_in[:])
            bseed = consts.tile([1, 2 * K], dt.float8e4)
            nc.sync.dma_start(bseed[:], bseed_in[:])
            bseed3 = bseed[:].rearrange("p (j n) -> p j n", j=2)
            ones = consts.tile([D, 1], dt.bfloat16)
            nc.sync.dma_start(ones[:], ones_in[:])

            out_sb = out_pool.tile([K, n_per_core * 132], dt.float32)
            # touch the ln/exp ACT table set immediately so its ~2.7us DMA
            # overlaps the initial input loads instead of the first chunk
            warm = consts.tile([1, 1], dt.float32)
            nc.scalar.activation(warm[:], ones[0:1, 0:1], AF.Ln)

            def emit_all():
              chunk_list = [
                  (n, ci) for n in range(n_read) for ci in range(n_chunks)
              ]
              lead_state = {}
              pv_state = {}

              def lead(n, ci):
                  """Per-chunk stats lead-in: load, square, per-unit sumsq
                  matmuls, rnorm. Emitted one chunk ahead of main() so the
                  baked in-order engine schedules interleave the next chunk's
                  lead-in with this chunk's softmax (no head-of-line block)."""
                  xc = xc_pool.tile([D, CHUNK], dt.bfloat16, name="xc")
                  # x is pre-cast to bf16 on the host (identical math to an
                  # on-device cast, half the HBM traffic). The very first
                  # chunk loads in quarters so compute starts ~2us sooner.
                  first_chunk = (n, ci) == (0, 0)
                  QC = CHUNK // 4
                  if first_chunk:
                      for q in range(4):
                          nc.gpsimd.dma_start(
                              xc[:, q * QC : (q + 1) * QC],
                              x_in[n, :, ci * CHUNK + q * QC
                                   : ci * CHUNK + (q + 1) * QC],
                          )
                  else:
                      nc.gpsimd.dma_start(
                          xc[:], x_in[n, :, ci * CHUNK : (ci + 1) * CHUNK]
                      )
                  rn = stats_pool.tile(
                      [128, units_per_chunk], dt.float32, tag="rn", name="rn"
                  )
                  lns = stats_pool.tile(
                      [128, units_per_chunk], dt.float32, tag="lns", name="lns"
                  )
                  # sumsq via PE: square x in natural layout, then per unit
                  # contract over d with a ones column, landing sumsq directly
                  # in s-partition orientation in PSUM.
                  xsq = scratch_pool.tile(
                      [D, CHUNK], dt.bfloat16, tag="xsq", name="xsq"
                  )
                  if first_chunk:
                      for q in range(4):
                          nc.vector.tensor_tensor(
                              out=xsq[:, q * QC : (q + 1) * QC],
                              in0=xc[:, q * QC : (q + 1) * QC],
                              in1=xc[:, q * QC : (q + 1) * QC], op=OP.mult,
                          )
                  elif ci % 4 == 3:
                      # balance: a quarter of the squares run on ACT
                      nc.scalar.activation(xsq[:], xc[:], AF.Square)
                  else:
                      nc.vector.tensor_tensor(
                          out=xsq[:], in0=xc[:], in1=xc[:], op=OP.mult
                      )
                  pss = pss_pool.tile([128, 96], dt.float32, name="pss")
                  ss = pss[:, 0:32]
                  for cu in range(units_per_chunk):
                      nc.tensor.matmul(
                          ss[:, cu : cu + 1],
                          xsq[:, cu * UNIT : (cu + 1) * UNIT],
                          ones[:], start=True, stop=True,
                      )
                  # rnorm = exp(-0.5*ln(sumsq)); stays inside one ACT table set
                  nc.scalar.activation(lns[:], ss[:], AF.Ln)
                  nc.scalar.activation(rn[:], lns[:], AF.Exp, scale=-0.5)
                  # one x^T tile per chunk; norm column (asum rhs) written now:
                  # norm = sqrt(sumsq) = exp(0.5*ln)
                  xt = xt_pool.tile(
                      [128, units_per_chunk * XT_STRIDE], dt.bfloat16, name="xt"
                  )
                  xt3 = xt[:].rearrange("p (u c) -> p u c", c=XT_STRIDE)
                  # fp8 seed row: norm = sqrt(ss) compact, PE-transposed to
                  # [32u, 128s], then partition-collapsed to one row so the
                  # per-unit rank-1 bias matmuls (norm[s]*b[k], DoubleRow)
                  # can use it as a base-partition-0 stationary
                  nr16 = stats_pool.tile(
                      [128, units_per_chunk], dt.bfloat16, tag="nr16",
                      name="nr16"
                  )
                  nc.scalar.activation(nr16[:], lns[:], AF.Exp, scale=0.5)
                  # asum column of xt copied from the compact norms on Pool
                  nc.gpsimd.tensor_copy(xt3[:, :, D : D + 1],
                                        nr16[:][:, :, None])
                  seedT = pss[0:32, 32:96].bitcast(dt.bfloat16)
                  nc.tensor.transpose(seedT, nr16[:], idm[:])
                  seed8 = seed_pool.tile(
                      [units_per_chunk, UNIT], dt.float8e4, tag="s8",
                      name="seed8"
                  )
                  nc.scalar.activation(seed8[:], seedT, AF.Copy)
                  seed8f = seed_pool.tile(
                      [1, units_per_chunk * UNIT], dt.float8e4, tag="s8f",
                      name="seed8f"
                  )
                  nc.sync.dma_start(seed8f[:], seed8[:])
                  lead_state[(n, ci)] = (xc, rn, xt, seed8f)

              def main(n, ci):
                  xc, rn, xt, seed8f = lead_state.pop((n, ci))
                  if ci == 0:
                      pv_state[n] = pv_pool.tile([K, 132], dt.float32, name="pv")
                  pv = pv_state[n]
                  xt3 = xt[:].rearrange("p (u c) -> p u c", c=XT_STRIDE)

                  psls = []
                  for h in halves:
                      psl = psl_pool.tile([128, HU * K], dt.float32, name="psl")
                      pst = pst_pool.tile([128, HU * D], dt.bfloat16, name="pst")
                      for u in range(HU):
                          cu = h * HU + u
                          lhsT = xc[:, cu * UNIT : (cu + 1) * UNIT]
                          srow = seed8f[0:1, cu * UNIT : (cu + 1) * UNIT][
                              :, None, :
                          ]
                          nc.tensor.matmul(
                              psl[:, u * K : (u + 1) * K],
                              srow.broadcast_to([1, 2, UNIT]), bseed3,
                              start=True, stop=False,
                              perf_mode=mybir.MatmulPerfMode.DoubleRow,
                          )
                          nc.tensor.matmul(
                              psl[:, u * K : (u + 1) * K], lhsT, wt[:],
                              start=False, stop=True,
                          )
                          nc.tensor.transpose(
                              pst[:, u * D : (u + 1) * D], lhsT, idm[:],
                          )
                      # batched PSUM->SBUF move of x^T (bf16), strided per unit
                      xt3h = xt3[:, h * HU : (h + 1) * HU, :]
                      pst3 = pst[:].rearrange("p (u c) -> p u c", c=D)
                      nc.scalar.activation(xt3h[:, :, 0:D], pst3, AF.Copy)
                      psls.append(psl)

                  for h in halves:
                      psl = psls[h]
                      rnh = rn[:, h * HU : (h + 1) * HU]
                      # l_scaled = logits_raw * rnorm (broadcast over k)
                      lsc = soft_pool.tile(
                          [128, HU * K], dt.bfloat16, tag="lsc", name="lsc"
                      )
                      nc.vector.tensor_tensor(
                          out=lsc[:].rearrange("p (u k) -> p u k", k=K),
                          in0=psl[:].rearrange("p (u k) -> p u k", k=K),
                          in1=rnh.broadcast_to([128, HU, K]),
                          op=OP.mult,
                      )
                      e = soft_pool.tile(
                          [128, HU * K], dt.bfloat16, tag="e", name="e"
                      )
                      nc.scalar.activation(e[:], lsc[:], AF.Exp)
                      # bias already folded in via the PE seed: e includes
                      # exp(b). Halve the reduce input with a 2x-mode bf16
                      # add first (tensor_reduce has no fast DVE mode).
                      e3 = e[:].rearrange("p (u k) -> p u k", k=K)
                      dnh = soft_pool.tile(
                          [128, HU * (K // 2)], dt.bfloat16, tag="dnh",
                          name="dnh"
                      )
                      nc.vector.tensor_tensor(
                          out=dnh[:].rearrange("p (u k) -> p u k", k=K // 2),
                          in0=e3[:, :, 0 : K // 2],
                          in1=e3[:, :, K // 2 : K], op=OP.add,
                      )
                      dn = stats_pool.tile([128, HU], dt.float32, tag="dn", name="dn")
                      nc.vector.tensor_reduce(
                          out=dn[:],
                          in_=dnh[:].rearrange("p (u k) -> p u k", k=K // 2),
                          axis=mybir.AxisListType.X, op=OP.add,
                      )
                      rdn = stats_pool.tile(
                          [128, HU], dt.float32, tag="rdn", name="rdn"
                      )
                      nc.vector.reciprocal(rdn[:], dn[:])
                      cc = stats_pool.tile([128, HU], dt.float32, tag="cc", name="cc")
                      nc.gpsimd.tensor_tensor(
                          out=cc[:], in0=rnh, in1=rdn[:], op=OP.mult
                      )
                      # a2 = a_un * (rnorm * rdenom)
                      a2 = soft_pool.tile(
                          [128, HU * K], dt.bfloat16, tag="a2", name="a2"
                      )
                      if OPT_SOFT == "gpsimd_ccb" or (
                          OPT_SOFT == "split" and h % 2 == 1
                      ):
                          # materialize cc broadcast (gpsimd 1-input) so the
                          # a2 multiply runs in the DVE 2x bf16 mode
                          ccb = soft_pool.tile(
                              [128, HU * K], dt.bfloat16, tag="ccb", name="ccb"
                          )
                          nc.gpsimd.tensor_copy(
                              ccb[:].rearrange("p (u k) -> p u k", k=K),
                              cc[:].broadcast_to([128, HU, K]),
                          )
                          nc.vector.tensor_tensor(
                              out=a2[:], in0=e[:], in1=ccb[:], op=OP.mult
                          )
                      else:
                          nc.vector.tensor_tensor(
                              out=a2[:].rearrange("p (u k) -> p u k", k=K),
                              in0=e[:].rearrange("p (u k) -> p u k", k=K),
                              in1=cc[:].broadcast_to([128, HU, K]),
                              op=OP.mult,
                          )
                      # VLAD accumulation: pv[:, :129] += a2_u^T @ [x^T | norm]
                      for u in range(HU):
                          cu = ci * units_per_chunk + h * HU + u
                          first = cu == 0
                          last = cu == (s_total // UNIT) - 1
                          xoff = (h * HU + u) * XT_STRIDE
                          nc.tensor.matmul(
                              pv[:, 0 : D + 1],
                              a2[:, u * K : (u + 1) * K],
                              xt[:, xoff : xoff + D + 1],
                              start=first, stop=last,
                          )
                  if ci == n_chunks - 1:
                      # stash [A | asum] for this n and ship it immediately
                      # so only the last image's store sits in the tail
                      nc.scalar.activation(
                          out_sb[:, n * 132 : n * 132 + D + 1],
                          pv[:, 0 : D + 1], AF.Copy,
                      )
                      nc.sync.dma_start(
                          out_dram[:, n * 132 : n * 132 + D + 1],
                          out_sb[:, n * 132 : n * 132 + D + 1],
                      )

              lead(*chunk_list[0])
              for i, (n, ci) in enumerate(chunk_list):
                  if i + 1 < len(chunk_list):
                      lead(*chunk_list[i + 1])
                  main(n, ci)
            if reps > 1:
                with tc.For_i(0, reps, 1):
                    emit_all()
            else:
                emit_all()

    _split_waits(nc, mybir)
    return nc


_CACHE = {}


def _get_program(n_per_core, s_total, reps=1, n_read=None):
    key = (n_per_core, s_total, reps, n_read)
    if key not in _CACHE:
        _CACHE[key] = build_program(n_per_core, s_total, reps, n_read)
    return _CACHE[key]


def run_device(x, conv_w, conv_b, n_per_core=4, s_total=S, trace=False):
    """Run the device part. x: [NCORES*n_per_core, D, s_total] fp32.
    Returns (A [n, K, D], asum [n, K], bass_results)."""
    from concourse.bass_utils import run_bass_kernel_spmd

    nc = _get_program(n_per_core, s_total)

    bf16 = ml_dtypes.bfloat16
    f8 = ml_dtypes.float8_e4m3
    wt_np = np.ascontiguousarray(conv_w.T.astype(bf16))           # [D, K]
    idm_np = np.eye(D, dtype=bf16)                                 # [D, D]
    bseed_np = np.concatenate(
        [conv_b.astype(np.float32), np.zeros(K, np.float32)]
    ).reshape(1, 2 * K).astype(f8)

    ones_np = np.ones((D, 1), bf16)
    in_maps = []
    for c in range(NCORES):
        xc = np.ascontiguousarray(
            x[c * n_per_core : (c + 1) * n_per_core].astype(bf16)
        )
        in_maps.append(
            {"x": xc, "wt": wt_np, "idm": idm_np, "bseed": bseed_np,
             "ones": ones_np}
        )

    try:
        res = run_bass_kernel_spmd(
            nc, in_maps, list(range(NCORES)), trace=trace,
        )
    except Exception:
        # one retry: the device occasionally reports a transient
        # unrecoverable state right after a failed prior load
        time.sleep(2)
        res = run_bass_kernel_spmd(
            nc, in_maps, list(range(NCORES)), trace=trace,
        )

    n_total = NCORES * n_per_core
    A = np.empty((n_total, K, D), np.float64)
    asum = np.empty((n_total, K), np.float64)
    for c in range(NCORES):
        o = res.results[c]["out"]  # [K, n_per_core*132]
        for nl in range(n_per_core):
            blk = o[:, nl * 132 : nl * 132 + D + 1].astype(np.float64)
            A[c * n_per_core + nl] = blk[:, :D]
            asum[c * n_per_core + nl] = blk[:, D]
    return A, asum, res


def finalize(A, asum, centroids, att_w, att_b):
    cen = centroids.astype(np.float64)
    vlad = A - asum[:, :, None] * cen[None]
    soft = cen @ att_w.astype(np.float64).T + att_b.astype(np.float64)  # [K, 1]
    av = vlad * soft[None]
    nrm = np.maximum(np.linalg.norm(av, axis=2, keepdims=True), EPS)
    return (av / nrm).astype(np.float32)


def kernel(x, conv_w, conv_b, centroids, att_w, att_b):
    x = np.asarray(x, np.float32)
    A, asum, _ = run_device(
        x, np.asarray(conv_w, np.float32), np.asarray(conv_b, np.float32)
    )
    return finalize(
        A, asum,
        np.asarray(centroids, np.float32),
        np.asarray(att_w, np.float32),
        np.asarray(att_b, np.float32),
    )


# revision 54
# speedup vs baseline: 1.0470x; 1.0082x over previous
"""AttVlad Trainium2 kernel.

Math (per image n):
  xn = x / ||x||_2(over d)                       x: [D=128, S]
  a  = softmax_k(conv_w @ xn + conv_b)           a: [K=64, S]
  vlad[k,d]   = sum_s a[k,s] xn[d,s] - (sum_s a[k,s]) centroids[k,d]
  out = normalize_d(vlad * (centroids @ att_w.T + att_b))

Device strategy (8 cores, data-parallel over n, 4 images each):
  - x is cast to bf16 on the host (the device math is bf16 either way, so
    this is bit-identical) and streamed in [128d, 4096s] HWDGE chunks,
    halving device HBM traffic.
  - Per 128-s unit: one PE pass with lhsT = x_chunk slice produces BOTH
    logits^T [128s, 64k] (rhs = conv_w^T) and x^T [128s, 128d] (rhs = I).
  - All per-s scalars (rsqrt of sumsq, softmax denom, their products) live
    as [128, 16] tiles (s on partitions) and are applied via broadcast
    (step-0) access patterns, so softmax needs no per-unit scalar ops.
  - Normalization scalars never touch x: logits are scaled by rnorm before
    exp; the VLAD matmul uses lhsT a2 = exp(l*rnorm)*exp(b) * (rnorm*rdenom)
    and rhs = [x^T | norm], giving columns [A | asum] accumulated in PSUM.
  - rsqrt is computed as exp(-0.5*ln(s)) to stay inside one ACT table set.
  - Host does the O(N*K*D) finalize (centroid subtract, attention scale,
    intra-normalization) in float64.
"""

import sys
import time

import numpy as np

try:  # the concourse stack (bass) ships in the container image
    import concourse.bass as _probe  # noqa: F401
except Exception:  # pragma: no cover
    sys.path.insert(0, "/opt/trn_rl_repo")

import ml_dtypes

N, D, S, K = 32, 128, 16384, 64
NCORES = 8
EPS = 1e-12

CHUNK = 4096  # s-positions per DMA chunk
UNIT = 128    # s-positions per matmul unit (psum partition dim)
XT_STRIDE = 130  # x^T unit stride in the SBUF tile: 128 cols x^T + 1 norm + 1 pad


def _make_tile_context_cls(tile, mybir, ScopedClock):
    """This walrus build rejects instructions carrying more than one sync
    wait; excess waits are split onto same-engine NoOps by _split_waits."""
    return tile.TileContext


# this walrus build rejects >1 sync wait on every instruction struct probed
# (CTRL, TT, MM); keep both caps at 1
MAX_WAITS = 1
COMPUTE_WAITS = 1
_COMPUTE_TYPES = (
    "InstTensorTensor", "InstActivation", "InstMatmult", "InstTensorReduce",
    "InstReciprocal", "InstTensorCopy", "InstLdweights", "InstTensorScalarPtr",
    "InstMemSet", "InstTensorScalar",
)


def _split_waits(nc, mybir):
    """Rewrite the traced BIR so no instruction carries more sem waits than
    this walrus build's per-struct limit: excess waits move to injected NoOps
    immediately preceding the instruction on the same engine (NX executes
    waits in order, so this is semantically identical)."""
    nid = 0
    for f in nc.m.functions:
        for blk in f.blocks:
            new_insts = []
            for inst in blk.instructions:
                si = getattr(inst, "sync_info", None)
                ws = list(si.on_wait) if si is not None else []
                maxw = (
                    COMPUTE_WAITS
                    if type(inst).__name__ in _COMPUTE_TYPES
                    else MAX_WAITS
                )
                if len(ws) > maxw:
                    extra = ws[: len(ws) - maxw]
                    for i in range(0, len(extra), MAX_WAITS):
                        nid += 1
                        nop = mybir.InstNoOp(
                            name=f"waitsplit_{nid}", ins=[], outs=[]
                        )
                        nop.engine = inst.engine
                        nop.sync_info = mybir.SyncInfo(
                            on_wait=extra[i : i + MAX_WAITS], on_update=[]
                        )
                        new_insts.append(nop)
                    si.on_wait = ws[len(ws) - maxw :]
                new_insts.append(inst)
            blk.instructions[:] = new_insts


# tunables (engine assignment variants, sweepable via the cost model)
OPT_SUMSQ = "pe"      # "dve": square+reduce on DVE | "pe": natural-layout square + ones-matmul
OPT_SOFT = "split"   # engine for a_un/a2 elementwise muls: "dve" | "gpsimd"


def build_program(n_per_core=4, s_total=S, reps=1, n_read=None):
    """Build the single-core Bass program (same program runs on all cores).
    reps>1 repeats the whole computation; n_read<n_per_core processes only
    the first n_read images (input shapes unchanged) — both are for
    slope-based HW timing."""
    if n_read is None:
        n_read = n_per_core
    import concourse.bass as bass
    import concourse.tile as tile
    from concourse import mybir
    from concourse.vector_clock import ScopedClock

    dt = mybir.dt
    AF = mybir.ActivationFunctionType
    OP = mybir.AluOpType

    TileContextFixed = _make_tile_context_cls(tile, mybir, ScopedClock)

    n_chunks = s_total // CHUNK
    units_per_chunk = CHUNK // UNIT
    HU = 8                                   # units per group (psum-bank sized)
    halves = tuple(range(units_per_chunk // HU))

    nc = bass.Bass()
    x_in = nc.declare_dram_parameter(
        "x", [n_per_core, D, s_total], dt.bfloat16, isOutput=False
    )
    wt_in = nc.declare_dram_parameter("wt", [D, K], dt.bfloat16, isOutput=False)
    idm_in = nc.declare_dram_parameter("idm", [D, D], dt.bfloat16, isOutput=False)
    bseed_in = nc.declare_dram_parameter(
        "bseed", [1, 2 * K], dt.float8e4, isOutput=False
    )
    ones_in = nc.declare_dram_parameter("ones", [D, 1], dt.bfloat16, isOutput=False)
    out_dram = nc.declare_dram_parameter(
        "out", [K, n_per_core * 132], dt.float32, isOutput=True
    )

    with TileContextFixed(nc) as tc:
        with (
            tc.tile_pool(name="consts", bufs=1) as consts,
            tc.tile_pool(name="xc", bufs=5) as xc_pool,
            tc.tile_pool(name="xt", bufs=5) as xt_pool,
            tc.tile_pool(name="soft", bufs=9) as soft_pool,
            tc.tile_pool(name="stats", bufs=8) as stats_pool,
            tc.tile_pool(name="scratch", bufs=4) as scratch_pool,
            tc.tile_pool(name="seed", bufs=4) as seed_pool,
            tc.tile_pool(name="outp", bufs=1) as out_pool,
            tc.tile_pool(name="psl", bufs=4, space="PSUM") as psl_pool,
            tc.tile_pool(name="pst", bufs=2, space="PSUM") as pst_pool,
            tc.tile_pool(name="pv", bufs=1, space="PSUM") as pv_pool,
            tc.tile_pool(name="pss", bufs=1, space="PSUM") as pss_pool,
        ):
            wt = consts.tile([D, K], dt.bfloat16)
            nc.sync.dma_start(wt[:], wt_in[:])
            idm = consts.tile([D, D], dt.bfloat16)
            nc.sync.dma_start(idm[:], idm_in[:])
            bseed = consts.tile([1, 2 * K], dt.float8e4)
            nc.sync.dma_start(bseed[:], bseed_in[:])
            bseed3 = bseed[:].rearrange("p (j n) -> p j n", j=2)
            ones = consts.tile([D, 1], dt.bfloat16)
            nc.sync.dma_start(ones[:], ones_in[:])

            out_sb = out_pool.tile([K, n_per_core * 132], dt.float32)
            # touch the ln/exp ACT table set immediately so its ~2.7us DMA
            # overlaps the initial input loads instead of the first chunk
            warm = consts.tile([1, 1], dt.float32)
            nc.scalar.activation(warm[:], ones[0:1, 0:1], AF.Ln)

            def emit_all():
              chunk_list = [
                  (n, ci) for n in range(n_read) for ci in range(n_chunks)
              ]
              lead_state = {}
              pv_state = {}

              def lead(n, ci):
                  """Per-chunk stats lead-in: load, square, per-unit sumsq
                  matmuls, rnorm. Emitted one chunk ahead of main() so the
                  baked in-order engine schedules interleave the next chunk's
                  lead-in with this chunk's softmax (no head-of-line block)."""
                  xc = xc_pool.tile([D, CHUNK], dt.bfloat16, name="xc")
                  # x is pre-cast to bf16 on the host (identical math to an
                  # on-device cast, half the HBM traffic). The very first
                  # chunk loads in quarters so compute starts ~2us sooner.
                  first_chunk = (n, ci) == (0, 0)
                  QC = CHUNK // 4
                  if first_chunk:
                      for q in range(4):
                          nc.gpsimd.dma_start(
                              xc[:, q * QC : (q + 1) * QC],
                              x_in[n, :, ci * CHUNK + q * QC
                                   : ci * CHUNK + (q + 1) * QC],
                          )
                  else:
                      nc.gpsimd.dma_start(
                          xc[:], x_in[n, :, ci * CHUNK : (ci + 1) * CHUNK]
                      )
                  rn = stats_pool.tile(
                      [128, units_per_chunk], dt.float32, tag="rn", name="rn"
                  )
                  lns = stats_pool.tile(
                      [128, units_per_chunk], dt.float32, tag="lns", name="lns"
                  )
                  # sumsq via PE: square x in natural layout, then per unit
                  # contract over d with a ones column, landing sumsq directly
                  # in s-partition orientation in PSUM.
                  xsq = scratch_pool.tile(
                      [D, CHUNK], dt.bfloat16, tag="xsq", name="xsq"
                  )
                  if first_chunk:
                      for q in range(4):
                          nc.vector.tensor_tensor(
                              out=xsq[:, q * QC : (q + 1) * QC],
                              in0=xc[:, q * QC : (q + 1) * QC],
                              in1=xc[:, q * QC : (q + 1) * QC], op=OP.mult,
                          )
                  elif ci % 4 == 3:
                      # balance: a quarter of the squares run on ACT
                      nc.scalar.activation(xsq[:], xc[:], AF.Square)
                  else:
                      nc.vector.tensor_tensor(
                          out=xsq[:], in0=xc[:], in1=xc[:], op=OP.mult
                      )
                  pss = pss_pool.tile([128, 96], dt.float32, name="pss")
                  ss = pss[:, 0:32]
                  for cu in range(units_per_chunk):
                      nc.tensor.matmul(
                          ss[:, cu : cu + 1],
                          xsq[:, cu * UNIT : (cu + 1) * UNIT],
                          ones[:], start=True, stop=True,
                      )
                  # rnorm = exp(-0.5*ln(sumsq)); stays inside one ACT table set
                  nc.scalar.activation(lns[:], ss[:], AF.Ln)
                  # one x^T tile per chunk; norm column (asum rhs) written now:
                  # norm = sqrt(sumsq) = exp(0.5*ln)
                  xt = xt_pool.tile(
                      [128, units_per_chunk * XT_STRIDE], dt.bfloat16, name="xt"
                  )
                  xt3 = xt[:].rearrange("p (u c) -> p u c", c=XT_STRIDE)
                  # fp8 seed row: norm = sqrt(ss) compact, PE-transposed to
                  # [32u, 128s], then partition-collapsed to one row so the
                  # per-unit rank-1 bias matmuls (norm[s]*b[k], DoubleRow)
                  # can use it as a base-partition-0 stationary
                  nr16 = stats_pool.tile(
                      [128, units_per_chunk], dt.bfloat16, tag="nr16",
                      name="nr16"
                  )
                  nc.scalar.activation(nr16[:], lns[:], AF.Exp, scale=0.5)
                  # asum column of xt copied from the compact norms on Pool
                  nc.gpsimd.tensor_copy(xt3[:, :, D : D + 1],
                                        nr16[:][:, :, None])
                  seedT = pss[0:32, 32:96].bitcast(dt.bfloat16)
                  nc.tensor.transpose(seedT, nr16[:], idm[:])
                  seed8 = seed_pool.tile(
                      [units_per_chunk, UNIT], dt.float8e4, tag="s8",
                      name="seed8"
                  )
                  nc.scalar.activation(seed8[:], seedT, AF.Copy)
                  nc.scalar.activation(rn[:], lns[:], AF.Exp, scale=-0.5)
                  seed8f = seed_pool.tile(
                      [1, units_per_chunk * UNIT], dt.float8e4, tag="s8f",
                      name="seed8f"
                  )
                  nc.sync.dma_start(seed8f[:], seed8[:])
                  lead_state[(n, ci)] = (xc, rn, xt, seed8f)

              def main(n, ci):
                  xc, rn, xt, seed8f = lead_state.pop((n, ci))
                  if ci == 0:
                      pv_state[n] = pv_pool.tile([K, 132], dt.float32, name="pv")
                  pv = pv_state[n]
                  xt3 = xt[:].rearrange("p (u c) -> p u c", c=XT_STRIDE)

                  psls = []
                  for h in halves:
                      psl = psl_pool.tile([128, HU * K], dt.float32, name="psl")
                      pst = pst_pool.tile([128, HU * D], dt.bfloat16, name="pst")
                      for u in range(HU):
                          cu = h * HU + u
                          lhsT = xc[:, cu * UNIT : (cu + 1) * UNIT]
                          srow = seed8f[0:1, cu * UNIT : (cu + 1) * UNIT][
                              :, None, :
                          ]
                          nc.tensor.matmul(
                              psl[:, u * K : (u + 1) * K],
                              srow.broadcast_to([1, 2, UNIT]), bseed3,
                              start=True, stop=False,
                              perf_mode=mybir.MatmulPerfMode.DoubleRow,
                          )
                          nc.tensor.matmul(
                              psl[:, u * K : (u + 1) * K], lhsT, wt[:],
                              start=False, stop=True,
                          )
                          nc.tensor.transpose(
                              pst[:, u * D : (u + 1) * D], lhsT, idm[:],
                          )
                      # batched PSUM->SBUF move of x^T (bf16), strided per unit
                      xt3h = xt3[:, h * HU : (h + 1) * HU, :]
                      pst3 = pst[:].rearrange("p (u c) -> p u c", c=D)
                      nc.scalar.activation(xt3h[:, :, 0:D], pst3, AF.Copy)
                      psls.append(psl)

                  for h in halves:
                      psl = psls[h]
                      rnh = rn[:, h * HU : (h + 1) * HU]
                      # l_scaled = logits_raw * rnorm (broadcast over k)
                      lsc = soft_pool.tile(
                          [128, HU * K], dt.bfloat16, tag="lsc", name="lsc"
                      )
                      nc.vector.tensor_tensor(
                          out=lsc[:].rearrange("p (u k) -> p u k", k=K),
                          in0=psl[:].rearrange("p (u k) -> p u k", k=K),
                          in1=rnh.broadcast_to([128, HU, K]),
                          op=OP.mult,
                      )
                      e = soft_pool.tile(
                          [128, HU * K], dt.bfloat16, tag="e", name="e"
                      )
                      nc.scalar.activation(e[:], lsc[:], AF.Exp)
                      # bias already folded in via the PE seed: e includes
                      # exp(b). Halve the reduce input with a 2x-mode bf16
                      # add first (tensor_reduce has no fast DVE mode).
                      e3 = e[:].rearrange("p (u k) -> p u k", k=K)
                      dnh = soft_pool.tile(
                          [128, HU * (K // 2)], dt.bfloat16, tag="dnh",
                          name="dnh"
                      )
                      nc.vector.tensor_tensor(
                          out=dnh[:].rearrange("p (u k) -> p u k", k=K // 2),
                          in0=e3[:, :, 0 : K // 2],
                          in1=e3[:, :, K // 2 : K], op=OP.add,
                      )
                      dn = stats_pool.tile([128, HU], dt.float32, tag="dn", name="dn")
                      nc.vector.tensor_reduce(
                          out=dn[:],
                          in_=dnh[:].rearrange("p (u k) -> p u k", k=K // 2),
                          axis=mybir.AxisListType.X, op=OP.add,
                      )
                      rdn = stats_pool.tile(
                          [128, HU], dt.float32, tag="rdn", name="rdn"
                      )
                      nc.vector.reciprocal(rdn[:], dn[:])
                      cc = stats_pool.tile([128, HU], dt.float32, tag="cc", name="cc")
                      nc.gpsimd.tensor_tensor(
                          out=cc[:], in0=rnh, in1=rdn[:], op=OP.mult
                      )
                      # a2 = a_un * (rnorm * rdenom)
                      a2 = soft_pool.tile(
                          [128, HU * K], dt.bfloat16, tag="a2", name="a2"
                      )
                      if OPT_SOFT == "gpsimd_ccb" or (
                          OPT_SOFT == "split" and h % 2 == 1
                      ):
                          # materialize cc broadcast (gpsimd 1-input) so the
                          # a2 multiply runs in the DVE 2x bf16 mode
                          ccb = soft_pool.tile(
                              [128, HU * K], dt.bfloat16, tag="ccb", name="ccb"
                          )
                          nc.gpsimd.tensor_copy(
                              ccb[:].rearrange("p (u k) -> p u k", k=K),
                              cc[:].broadcast_to([128, HU, K]),
                          )
                          nc.vector.tensor_tensor(
                              out=a2[:], in0=e[:], in1=ccb[:], op=OP.mult
                          )
                      else:
                          nc.vector.tensor_tensor(
                              out=a2[:].rearrange("p (u k) -> p u k", k=K),
                              in0=e[:].rearrange("p (u k) -> p u k", k=K),
                              in1=cc[:].broadcast_to([128, HU, K]),
                              op=OP.mult,
                          )
                      # VLAD accumulation: pv[:, :129] += a2_u^T @ [x^T | norm]
                      for u in range(HU):
                          cu = ci * units_per_chunk + h * HU + u
                          first = cu == 0
                          last = cu == (s_total // UNIT) - 1
                          xoff = (h * HU + u) * XT_STRIDE
                          nc.tensor.matmul(
                              pv[:, 0 : D + 1],
                              a2[:, u * K : (u + 1) * K],
                              xt[:, xoff : xoff + D + 1],
                              start=first, stop=last,
                          )
                  if ci == n_chunks - 1:
                      # stash [A | asum] for this n and ship it immediately
                      # so only the last image's store sits in the tail
                      nc.scalar.activation(
                          out_sb[:, n * 132 : n * 132 + D + 1],
                          pv[:, 0 : D + 1], AF.Copy,
                      )
                      nc.sync.dma_start(
                          out_dram[:, n * 132 : n * 132 + D + 1],
                          out_sb[:, n * 132 : n * 132 + D + 1],
                      )

              lead(*chunk_list[0])
              for i, (n, ci) in enumerate(chunk_list):
                  if i + 1 < len(chunk_list):
                      lead(*chunk_list[i + 1])
                  main(n, ci)
            if reps > 1:
                with tc.For_i(0, reps, 1):
                    emit_all()
            else:
                emit_all()

    _split_waits(nc, mybir)
    return nc


_CACHE = {}


def _get_program(n_per_core, s_total, reps=1, n_read=None):
    key = (n_per_core, s_total, reps, n_read)
    if key not in _CACHE:
        _CACHE[key] = build_program(n_per_core, s_total, reps, n_read)
    return _CACHE[key]


def run_device(x, conv_w, conv_b, n_per_core=4, s_total=S, trace=False):
    """Run the device part. x: [NCORES*n_per_core, D, s_total] fp32.
    Returns (A [n, K, D], asum [n, K], bass_results)."""
    from concourse.bass_utils import run_bass_kernel_spmd

    nc = _get_program(n_per_core, s_total)

    bf16 = ml_dtypes.bfloat16
    f8 = ml_dtypes.float8_e4m3
    wt_np = np.ascontiguousarray(conv_w.T.astype(bf16))           # [D, K]
    idm_np = np.eye(D, dtype=bf16)                                 # [D, D]
    bseed_np = np.concatenate(
        [conv_b.astype(np.float32), np.zeros(K, np.float32)]
    ).reshape(1, 2 * K).astype(f8)

    ones_np = np.ones((D, 1), bf16)
    in_maps = []
    for c in range(NCORES):
        xc = np.ascontiguousarray(
            x[c * n_per_core : (c + 1) * n_per_core].astype(bf16)
        )
        in_maps.append(
            {"x": xc, "wt": wt_np, "idm": idm_np, "bseed": bseed_np,
             "ones": ones_np}
        )

    try:
        res = run_bass_kernel_spmd(
            nc, in_maps, list(range(NCORES)), trace=trace,
        )
    except Exception:
        # one retry: the device occasionally reports a transient
        # unrecoverable state right after a failed prior load
        time.sleep(2)
        res = run_bass_kernel_spmd(
            nc, in_maps, list(range(NCORES)), trace=trace,
        )

    n_total = NCORES * n_per_core
    A = np.empty((n_total, K, D), np.float64)
    asum = np.empty((n_total, K), np.float64)
    for c in range(NCORES):
        o = res.results[c]["out"]  # [K, n_per_core*132]
        for nl in range(n_per_core):
            blk = o[:, nl * 132 : nl * 132 + D + 1].astype(np.float64)
            A[c * n_per_core + nl] = blk[:, :D]
            asum[c * n_per_core + nl] = blk[:, D]
    return A, asum, res


def finalize(A, asum, centroids, att_w, att_b):
    cen = centroids.astype(np.float64)
    vlad = A - asum[:, :, None] * cen[None]
    soft = cen @ att_w.astype(np.float64).T + att_b.astype(np.float64)  # [K, 1]
    av = vlad * soft[None]
    nrm = np.maximum(np.linalg.norm(av, axis=2, keepdims=True), EPS)
    return (av / nrm).astype(np.float32)


def kernel(x, conv_w, conv_b, centroids, att_w, att_b):
    x = np.asarray(x, np.float32)
    A, asum, _ = run_device(
        x, np.asarray(conv_w, np.float32), np.asarray(conv_b, np.float32)
    )
    return finalize(
        A, asum,
        np.asarray(centroids, np.float32),
        np.asarray(att_w, np.float32),
        np.asarray(att_b, np.float32),
    )
